# revision 3
# baseline (speedup 1.0000x reference)
"""Trainium2 Bass kernel for nn_Mlp_cnn_shift (dense CNN MLP with 3x3 patch-shift
and a softmax-gated mix of two branches).

Strategy
--------
Data-parallel over the 16 (B,T) frames: each of the 8 NeuronCores processes 2
frames end-to-end.  All activations are kept channel-major ([C, tokens]) so the
channel contraction of every matmul has K on partitions, and `x` is
pre-transposed/cast on the host so no on-device transpose is needed.

The patch-shift is handled by storing xh ([HID, tokens]) in a zero-padded token
layout: row pitch 57 (56 cols + 1 zero pad col) with 58-token zero guards on
both ends of each frame.  Every one of the 9 (dh, dw) rolls then becomes a pure
offset into the token axis of the fc1 matmul's rhs access pattern, with the
zero padding reproducing the reference's zero-fill boundary semantics exactly.
The 9 channel groups (114 wide for HID, 57 wide for C) are aligned to
128-partition blocks by permuting/padding the weight matrices on the host.

The only cross-core coupling is the global (T,H,W) mean feeding the softmax
gate: a [2,128,4] f32 AllReduce (~4KB).

Everything runs in one kernel launch per chip; bf16 matmuls with f32 PSUM
accumulation; output is f32.
"""

import os
import sys

for _p in ("/opt/trn_rl_repo",):
    if os.path.isdir(_p) and _p not in sys.path:
        sys.path.append(_p)

import numpy as np
import ml_dtypes

import concourse.bass as bass
import concourse.mybir as mybir
import concourse.tile as tile
from concourse import bacc
from concourse.bass_utils import run_bass_kernel_spmd

# ---------------------------------------------------------------- constants
SHIFTS = [(1, 1), (1, 0), (1, -1), (0, 1), (0, 0), (0, -1), (-1, 1), (-1, 0), (-1, -1)]
NG = 9
B, T, H, W, C = 2, 8, 56, 56, 512
HID = 1024
NCORES = 8
NF = (B * T) // NCORES          # frames per core = 2
HWTOK = H * W                   # 3136 tokens per frame
RP = W + 1                      # padded row pitch = 57
GUARD = RP + 1                  # 58 zero tokens on each end
FRPAD = RP * H                  # 3192
XHSPAN = GUARD + FRPAD + GUARD  # 3308
RG = 7                          # row groups per frame
RGR = H // RG                   # 8 rows per group
RGT = RGR * W                   # 448 valid tokens per row group
RGP = RGR * RP                  # 456 padded tokens per row group
GS_HID = 114                    # ceil(1026/9): hid shift-group size
GS_C = 57                       # ceil(513/9): C shift-group size
CCB = C // 128                  # 4 channel blocks
HCB = HID // 128                # 8
MEAN_N = float(T * H * W)       # the reference mean is over (T,H,W)

F32 = mybir.dt.float32
BF16 = mybir.dt.bfloat16
BF16_NP = ml_dtypes.bfloat16

_CACHE = {}


def _cpieces(c0, c1):
    """Split channel range [c0,c1) into (cblock, p0, p1) partition pieces."""
    out = []
    c = c0
    while c < c1:
        cb, p0 = divmod(c, 128)
        p1 = min(128, p0 + (c1 - c))
        out.append((cb, p0, p1))
        c += p1 - p0
    return out


# ---------------------------------------------------------------- device kernel
def build_nc():
    nc = bacc.Bacc("TRN2", target_bir_lowering=False, debug=False, num_devices=NCORES)

    dp = nc.declare_dram_parameter
    xT = dp("xT", [NF, 128, CCB, HWTOK], BF16, isOutput=False)
    fcw = dp("fcw", [128, CCB, NG * 128], BF16, isOutput=False)
    fcb = dp("fcb", [128, NG], F32, isOutput=False)
    fc1w = dp("fc1w", [128, NG, C], BF16, isOutput=False)
    fc1b = dp("fc1b", [128, CCB], F32, isOutput=False)
    fc2w = dp("fc2w", [128, NG, C], BF16, isOutput=False)
    fc2b = dp("fc2b", [128, CCB], F32, isOutput=False)
    projw = dp("projw", [128, CCB, C], BF16, isOutput=False)
    projb = dp("projb", [128, C], F32, isOutput=False)
    rw1w = dp("rw1w", [128, CCB, 128], BF16, isOutput=False)
    rw1b = dp("rw1b", [128, 1], F32, isOutput=False)
    rw2w = dp("rw2w", [128, 2 * C], BF16, isOutput=False)
    rw2b = dp("rw2b", [128, HCB], F32, isOutput=False)
    bmask = dp("bmask", [128, B], F32, isOutput=False)
    out_d = dp("out", [NF, HWTOK, C], F32, isOutput=True)

    # spill space for frame 0's h/w branches + collective bounce buffers
    h0d = nc.dram_tensor("h0d", [128, CCB, HWTOK], BF16)
    w0d = nc.dram_tensor("w0d", [128, CCB, HWTOK], BF16)
    ccin = nc.dram_tensor("ccin", [B, 128, CCB], F32)
    ccout = nc.dram_tensor("ccout", [B, 128, CCB], F32, addr_space="Shared")

    AF = mybir.ActivationFunctionType
    ALU = mybir.AluOpType

    with tile.TileContext(nc, num_cores=NCORES) as tc:
        with (
            tc.tile_pool(name="singles", bufs=1) as singles,
            tc.tile_pool(name="xh_pool", bufs=1) as xh_pool,
            tc.tile_pool(name="y_pool", bufs=1) as y_pool,
            tc.tile_pool(name="h_pool", bufs=1) as h_pool,
            tc.tile_pool(name="w_pool", bufs=1) as w_pool,
            tc.tile_pool(name="xt_pool", bufs=2) as xt_pool,
            tc.tile_pool(name="ostage", bufs=3) as ostage,
            tc.tile_pool(name="dstream", bufs=2) as dstream,
            tc.tile_pool(name="small", bufs=1) as small,
            tc.tile_pool(name="mmpsum", bufs=6, space="PSUM") as mmpsum,
            tc.tile_pool(name="gpsum", bufs=1, space="PSUM") as gpsum,
        ):
            # ---- load weights (resident for the whole kernel)
            fcw_s = singles.tile([128, CCB, NG * 128], BF16)
            nc.sync.dma_start(out=fcw_s, in_=fcw[:])
            fcb_s = singles.tile([128, NG], F32)
            nc.sync.dma_start(out=fcb_s, in_=fcb[:])
            fc1w_s = singles.tile([128, NG, C], BF16)
            nc.sync.dma_start(out=fc1w_s, in_=fc1w[:])
            fc1b_s = singles.tile([128, CCB], F32)
            nc.sync.dma_start(out=fc1b_s, in_=fc1b[:])
            fc2w_s = singles.tile([128, NG, C], BF16)
            nc.sync.dma_start(out=fc2w_s, in_=fc2w[:])
            fc2b_s = singles.tile([128, CCB], F32)
            nc.sync.dma_start(out=fc2b_s, in_=fc2b[:])
            projw_s = singles.tile([128, CCB, C], BF16)
            nc.sync.dma_start(out=projw_s, in_=projw[:])
            projb_s = singles.tile([128, C], F32)
            nc.sync.dma_start(out=projb_s, in_=projb[:])
            rw1w_s = singles.tile([128, CCB, 128], BF16)
            nc.sync.dma_start(out=rw1w_s, in_=rw1w[:])
            rw1b_s = singles.tile([128, 1], F32)
            nc.sync.dma_start(out=rw1b_s, in_=rw1b[:])
            rw2w_s = singles.tile([128, 2 * C], BF16)
            nc.sync.dma_start(out=rw2w_s, in_=rw2w[:])
            rw2b_s = singles.tile([128, HCB], F32)
            nc.sync.dma_start(out=rw2b_s, in_=rw2b[:])
            bmask_s = singles.tile([128, B], F32)
            nc.sync.dma_start(out=bmask_s, in_=bmask[:])

            tot_s = singles.tile([128, CCB], F32)     # sum over local frames of (h+w)
            a0_s = singles.tile([128, CCB], F32)      # gate for the h branch

            # xh, padded token layout, persistent across frames.
            xh = xh_pool.tile([128, NG, XHSPAN], BF16)
            # zero the guards and the per-row pad column once; the body is
            # fully rewritten by every frame's fc pass.
            nc.vector.memset(xh[:, :, :GUARD], 0.0)
            nc.vector.memset(xh[:, :, GUARD + FRPAD:], 0.0)
            xh_rows = xh[:, :, GUARD:GUARD + FRPAD].rearrange(
                "p g (r c) -> p g r c", c=RP
            )
            nc.vector.memset(xh_rows[:, :, :, W:], 0.0)

            hw_tiles = []  # (h_tile, w_tile) per frame; only f=1's stay valid

            for f in range(NF):
                # ---------------- A: xh = gelu(x @ fc_w + fc_b), group-blocked
                for rg in range(RG):
                    xt_t = xt_pool.tile([128, CCB, RGT], BF16, tag="xt")
                    nc.sync.dma_start(
                        out=xt_t, in_=xT[f, :, :, rg * RGT:(rg + 1) * RGT]
                    )
                    for mb in range(NG):
                        ps = mmpsum.tile([128, 512], F32, tag="mm")
                        for k in range(CCB):
                            nc.tensor.matmul(
                                ps[:, :RGT],
                                lhsT=fcw_s[:, k, mb * 128:(mb + 1) * 128],
                                rhs=xt_t[:, k, :],
                                start=(k == 0),
                                stop=(k == CCB - 1),
                            )
                        dst = xh[
                            :, mb, GUARD + rg * RGP:GUARD + (rg + 1) * RGP
                        ].rearrange("p (r c) -> p r c", c=RP)[:, :, :W]
                        src = ps[:, :RGT].rearrange("p (r c) -> p r c", c=W)
                        nc.scalar.activation(
                            out=dst, in_=src, func=AF.Gelu,
                            bias=fcb_s[:, mb:mb + 1],
                        )

                # ---------------- C: y = gelu(shift(xh) @ fc1_w + fc1_b)
                y = y_pool.tile([128, CCB, HWTOK], BF16, tag="y")
                for rg in range(RG):
                    for mb in range(CCB):
                        ps = mmpsum.tile([128, 512], F32, tag="mm")
                        for g in range(NG):
                            off = -(SHIFTS[g][0] * RP + SHIFTS[g][1])
                            s0 = GUARD + rg * RGP + off
                            nc.tensor.matmul(
                                ps[:, :RGP],
                                lhsT=fc1w_s[:, g, mb * 128:(mb + 1) * 128],
                                rhs=xh[:, g, s0:s0 + RGP],
                                start=(g == 0),
                                stop=(g == NG - 1),
                            )
                        dst = y[:, mb, rg * RGT:(rg + 1) * RGT].rearrange(
                            "p (r c) -> p r c", c=W
                        )
                        src = ps[:, :RGP].rearrange("p (r c) -> p r c", c=RP)[:, :, :W]
                        nc.scalar.activation(
                            out=dst, in_=src, func=AF.Gelu,
                            bias=fc1b_s[:, mb:mb + 1],
                        )

                # h = inverse-shift(y): h(i,j)[c in grp g] = y(i+sh, j+sw)[c]
                h_t = h_pool.tile([128, CCB, HWTOK], BF16, tag="h")
                nc.gpsimd.memset(h_t[:], 0.0)
                h4 = h_t.rearrange("p c (i j) -> p c i j", j=W)
                y4 = y.rearrange("p c (i j) -> p c i j", j=W)
                for g in range(NG):
                    sh, sw = SHIFTS[g]
                    i0, i1 = max(0, -sh), min(H, H - sh)
                    j0, j1 = max(0, -sw), min(W, W - sw)
                    for (cb, p0, p1) in _cpieces(GS_C * g, min(GS_C * (g + 1), C)):
                        # DMA (not DVE): compute engines need 32-aligned
                        # partition bases; DMA is address-based.
                        nc.sync.dma_start(
                            out=h4[p0:p1, cb, i0:i1, j0:j1],
                            in_=y4[p0:p1, cb, i0 + sh:i1 + sh, j0 + sw:j1 + sw],
                        )

                # ---------------- B: w = gelu(xh @ fc2_w + fc2_b)
                w_t = w_pool.tile([128, CCB, HWTOK], BF16, tag="w")
                for rg in range(RG):
                    for mb in range(CCB):
                        ps = mmpsum.tile([128, 512], F32, tag="mm")
                        for g in range(NG):
                            s0 = GUARD + rg * RGP
                            nc.tensor.matmul(
                                ps[:, :RGP],
                                lhsT=fc2w_s[:, g, mb * 128:(mb + 1) * 128],
                                rhs=xh[:, g, s0:s0 + RGP],
                                start=(g == 0),
                                stop=(g == NG - 1),
                            )
                        dst = w_t[:, mb, rg * RGT:(rg + 1) * RGT].rearrange(
                            "p (r c) -> p r c", c=W
                        )
                        src = ps[:, :RGP].rearrange("p (r c) -> p r c", c=RP)[:, :, :W]
                        nc.scalar.activation(
                            out=dst, in_=src, func=AF.Gelu,
                            bias=fc2b_s[:, mb:mb + 1],
                        )

                # ---------------- token sums for the gate mean
                hs = small.tile([128, CCB], F32, tag=f"hs{f}")
                nc.vector.tensor_reduce(
                    out=hs, in_=h_t[:], axis=mybir.AxisListType.X, op=ALU.add
                )
                ws = small.tile([128, CCB], F32, tag=f"ws{f}")
                nc.vector.tensor_reduce(
                    out=ws, in_=w_t[:], axis=mybir.AxisListType.X, op=ALU.add
                )
                if f == 0:
                    nc.vector.tensor_tensor(tot_s, hs, ws, ALU.add)
                else:
                    nc.vector.tensor_tensor(tot_s, tot_s, hs, ALU.add)
                    nc.vector.tensor_tensor(tot_s, tot_s, ws, ALU.add)

                if f == 0:
                    # spill frame 0's branches; streamed back during D0
                    nc.sync.dma_start(out=h0d[:], in_=h_t[:])
                    nc.sync.dma_start(out=w0d[:], in_=w_t[:])
                hw_tiles.append((h_t, w_t))

            # ---------------- cross-core AllReduce of the per-batch sums
            t0 = small.tile([128, CCB], F32, tag="cc0")
            nc.vector.tensor_scalar_mul(t0, tot_s, bmask_s[:, 0:1])
            t1 = small.tile([128, CCB], F32, tag="cc1")
            nc.vector.tensor_scalar_mul(t1, tot_s, bmask_s[:, 1:2])
            nc.sync.dma_start(out=ccin[0], in_=t0)
            nc.sync.dma_start(out=ccin[1], in_=t1)
            nc.gpsimd.collective_compute(
                "AllReduce",
                ALU.add,
                replica_groups=[list(range(NCORES))],
                ins=[ccin[:]],
                outs=[ccout[:]],
            )
            za = small.tile([128, CCB], F32, tag="za")
            nc.sync.dma_start(out=za, in_=ccout[0])
            zb = small.tile([128, CCB], F32, tag="zb")
            nc.sync.dma_start(out=zb, in_=ccout[1])
            nc.vector.tensor_scalar_mul(za, za, bmask_s[:, 0:1])
            nc.vector.tensor_scalar_mul(zb, zb, bmask_s[:, 1:2])
            zsum = small.tile([128, CCB], F32, tag="zsum")
            nc.vector.tensor_tensor(zsum, za, zb, ALU.add)
            zbf = small.tile([128, CCB], BF16, tag="zbf")
            nc.vector.tensor_copy(out=zbf, in_=zsum)

            # ---------------- gate: a = softmax over the 2 streams
            # (1/MEAN_N is folded into rw1w on the host)
            psg = gpsum.tile([128, 1], F32, tag="psg")
            for k in range(CCB):
                nc.tensor.matmul(
                    psg,
                    lhsT=rw1w_s[:, k, :],
                    rhs=zbf[:, k:k + 1],
                    start=(k == 0),
                    stop=(k == CCB - 1),
                )
            gv = small.tile([128, 1], BF16, tag="gv")
            nc.scalar.activation(
                out=gv, in_=psg, func=AF.Gelu, bias=rw1b_s[:, 0:1]
            )
            psu = gpsum.tile([128, HCB], F32, tag="psu")
            for m in range(HCB):
                nc.tensor.matmul(
                    psu[:, m:m + 1],
                    lhsT=rw2w_s[:, m * 128:(m + 1) * 128],
                    rhs=gv,
                    start=True,
                    stop=True,
                )
            uv = small.tile([128, HCB], F32, tag="uv")
            nc.vector.tensor_tensor(uv, psu, rw2b_s, ALU.add)
            # rw2 columns were permuted on host: cols [0:512] are the stream-0
            # logits in channel-major order, [512:1024] stream-1.
            l0, l1 = uv[:, 0:CCB], uv[:, CCB:2 * CCB]
            mx = small.tile([128, CCB], F32, tag="mx")
            nc.vector.tensor_tensor(mx, l0, l1, ALU.max)
            d0 = small.tile([128, CCB], F32, tag="d0")
            nc.vector.tensor_tensor(d0, l0, mx, ALU.subtract)
            d1 = small.tile([128, CCB], F32, tag="d1")
            nc.vector.tensor_tensor(d1, l1, mx, ALU.subtract)
            e0 = small.tile([128, CCB], F32, tag="e0")
            nc.scalar.activation(out=e0, in_=d0, func=AF.Exp)
            e1 = small.tile([128, CCB], F32, tag="e1")
            nc.scalar.activation(out=e1, in_=d1, func=AF.Exp)
            esum = small.tile([128, CCB], F32, tag="esum")
            nc.vector.tensor_tensor(esum, e0, e1, ALU.add)
            rec = small.tile([128, CCB], F32, tag="rec")
            nc.vector.reciprocal(rec, esum)
            nc.vector.tensor_tensor(a0_s, e0, rec, ALU.mult)

            # ---------------- D: out = (a0*h + (1-a0)*w) @ proj_w + proj_b
            def proj_blocks(gated_ap, fidx, tbase, ntok):
                """gated_ap: [128, CCB, ntok] bf16 SBUF ap (channel-major)."""
                m0 = 0
                while m0 < ntok:
                    M = min(128, ntok - m0)
                    pp = mmpsum.tile([128, 512], F32, tag="mm")
                    for cb in range(CCB):
                        nc.tensor.matmul(
                            pp[:M, :C],
                            lhsT=gated_ap[:, cb, m0:m0 + M],
                            rhs=projw_s[:, cb, :],
                            start=(cb == 0),
                            stop=(cb == CCB - 1),
                        )
                    ot = ostage.tile([128, C], F32, tag="ot")
                    nc.vector.tensor_tensor(ot[:M], pp[:M, :C], projb_s[:M], ALU.add)
                    nc.sync.dma_start(
                        out=out_d[fidx, tbase + m0:tbase + m0 + M, :], in_=ot[:M]
                    )
                    m0 += M

            def gate_inplace(h_ap, w_ap, ntok):
                """h_ap <- a0*h + (1-a0)*w   (= w + a0*(h-w)), in place."""
                nc.vector.tensor_tensor(h_ap, h_ap, w_ap, ALU.subtract)
                for cb in range(CCB):
                    nc.scalar.activation(
                        out=h_ap[:, cb, :], in_=h_ap[:, cb, :],
                        func=AF.Copy, scale=a0_s[:, cb:cb + 1],
                    )
                nc.vector.tensor_tensor(h_ap, h_ap, w_ap, ALU.add)

            # frame 1 from SBUF
            h1, w1 = hw_tiles[1]
            gate_inplace(h1[:], w1[:], HWTOK)
            proj_blocks(h1, 1, 0, HWTOK)

            # frame 0 streamed back from DRAM in 512-token chunks
            ck0 = 0
            while ck0 < HWTOK:
                CK = min(512, HWTOK - ck0)
                hc = dstream.tile([128, CCB, 512], BF16, tag="hc")
                nc.sync.dma_start(out=hc[:, :, :CK], in_=h0d[:, :, ck0:ck0 + CK])
                wc = dstream.tile([128, CCB, 512], BF16, tag="wc")
                nc.sync.dma_start(out=wc[:, :, :CK], in_=w0d[:, :, ck0:ck0 + CK])
                gate_inplace(hc[:, :, :CK], wc[:, :, :CK], CK)
                proj_blocks(hc, 0, ck0, CK)
                ck0 += CK

    nc.compile()
    return nc


# ---------------------------------------------------------------- host side
def _prep_weights(fc_w, fc_b, fc1_w, fc1_b, fc2_w, fc2_b,
                  rw1_w, rw1_b, rw2_w, rw2_b, proj_w, proj_b):
    f32 = np.float32

    # fc: columns permuted into 9 groups of 114 (112 for g=8), each padded to 128
    fcwp = np.zeros((C, NG * 128), f32)
    fcbp = np.zeros((NG * 128,), f32)
    for g in range(NG):
        n = min(GS_HID * (g + 1), HID) - GS_HID * g
        fcwp[:, 128 * g:128 * g + n] = fc_w[:, GS_HID * g:GS_HID * g + n]
        fcbp[128 * g:128 * g + n] = fc_b[GS_HID * g:GS_HID * g + n]
    fcw_h = np.ascontiguousarray(
        fcwp.reshape(CCB, 128, NG * 128).transpose(1, 0, 2)
    ).astype(BF16_NP)
    fcb_h = np.ascontiguousarray(fcbp.reshape(NG, 128).T).astype(f32)

    def rows_grouped(wm):  # [HID, C] -> [128, NG, C] with padded group rows
        wp = np.zeros((NG * 128, C), f32)
        for g in range(NG):
            n = min(GS_HID * (g + 1), HID) - GS_HID * g
            wp[128 * g:128 * g + n] = wm[GS_HID * g:GS_HID * g + n]
        return np.ascontiguousarray(
            wp.reshape(NG, 128, C).transpose(1, 0, 2)
        ).astype(BF16_NP)

    fc1w_h = rows_grouped(fc1_w)
    fc2w_h = rows_grouped(fc2_w)
    fc1b_h = np.ascontiguousarray(fc1_b.reshape(CCB, 128).T).astype(f32)
    fc2b_h = np.ascontiguousarray(fc2_b.reshape(CCB, 128).T).astype(f32)

    projw_h = np.ascontiguousarray(
        proj_w.reshape(CCB, 128, C).transpose(1, 0, 2)
    ).astype(BF16_NP)
    projb_h = np.ascontiguousarray(
        np.broadcast_to(proj_b[None, :], (128, C))
    ).astype(f32)

    rw1w_h = np.ascontiguousarray(
        (rw1_w / MEAN_N).reshape(CCB, 128, C // 4 // 1).transpose(1, 0, 2)
    ).astype(BF16_NP)
    rw1b_h = np.ascontiguousarray(rw1_b[:, None]).astype(f32)

    # rw2 columns: evens (stream 0) then odds (stream 1), channel-major
    rw2p = np.concatenate([rw2_w[:, 0::2], rw2_w[:, 1::2]], axis=1)
    rw2w_h = np.ascontiguousarray(rw2p).astype(BF16_NP)
    rw2bp = np.concatenate([rw2_b[0::2], rw2_b[1::2]])
    rw2b_h = np.ascontiguousarray(rw2bp.reshape(HCB, 128).T).astype(f32)

    return dict(
        fcw=fcw_h, fcb=fcb_h, fc1w=fc1w_h, fc1b=fc1b_h, fc2w=fc2w_h,
        fc2b=fc2b_h, projw=projw_h, projb=projb_h, rw1w=rw1w_h, rw1b=rw1b_h,
        rw2w=rw2w_h, rw2b=rw2b_h,
    )


def _get_nc():
    if "nc" not in _CACHE:
        _CACHE["nc"] = build_nc()
    return _CACHE["nc"]


def run(inputs, trace=False, trace_kwargs=None):
    """Run the SPMD kernel; returns (full_output, BassKernelResults)."""
    x = np.asarray(inputs["x"], np.float32)
    shared = _prep_weights(
        np.asarray(inputs["fc_w"], np.float32), np.asarray(inputs["fc_b"], np.float32),
        np.asarray(inputs["fc1_w"], np.float32), np.asarray(inputs["fc1_b"], np.float32),
        np.asarray(inputs["fc2_w"], np.float32), np.asarray(inputs["fc2_b"], np.float32),
        np.asarray(inputs["rw1_w"], np.float32), np.asarray(inputs["rw1_b"], np.float32),
        np.asarray(inputs["rw2_w"], np.float32), np.asarray(inputs["rw2_b"], np.float32),
        np.asarray(inputs["proj_w"], np.float32), np.asarray(inputs["proj_b"], np.float32),
    )

    xf = x.reshape(B * T, HWTOK, C)
    in_maps = []
    for c in range(NCORES):
        sh = xf[NF * c:NF * (c + 1)]                      # [NF, 3136, 512]
        xt = sh.transpose(0, 2, 1).reshape(NF, CCB, 128, HWTOK)
        xt = np.ascontiguousarray(xt.transpose(0, 2, 1, 3)).astype(BF16_NP)
        bm = np.zeros((128, B), np.float32)
        bm[:, (NF * c) // T] = 1.0
        m = dict(shared)
        m["xT"] = xt
        m["bmask"] = bm
        in_maps.append(m)

    nc = _get_nc()
    res = run_bass_kernel_spmd(
        nc, in_maps, list(range(NCORES)),
        trace=trace, **(dict(trace_kwargs=trace_kwargs) if trace_kwargs else {}),
    )

    out = np.empty((B * T, HWTOK, C), np.float32)
    for c in range(NCORES):
        out[NF * c:NF * (c + 1)] = res.results[c]["out"]
    return out.reshape(B, T, H, W, C), res


def kernel(**inputs) -> np.ndarray:
    full, _ = run(inputs, trace=False)
    return full


# revision 8
# speedup vs baseline: 1.1350x; 1.1350x over previous
"""Trainium2 Bass kernel for nn_Mlp_cnn_shift (dense CNN MLP with 3x3 patch-shift
and a softmax-gated mix of two branches).

Strategy
--------
Data-parallel over the 16 (B,T) frames: each of the 8 NeuronCores processes 2
frames end-to-end.  All activations are kept channel-major ([C, tokens]) so the
channel contraction of every matmul has K on partitions, and `x` is
pre-transposed/cast on the host so no on-device transpose is needed.

Patch-shift handling:
 * forward shift (on xh, HID=1024): xh is stored in a zero-padded token layout
   (row pitch 57 = 56 cols + 1 zero pad col, 58-token zero guards per frame)
   and in 9 channel groups of 114 padded to 128 partitions each (host-permuted
   fc_w columns / fc1_w+fc2_w rows).  Every (dh,dw) roll then becomes a pure
   token offset in the fc1 matmul's rhs access pattern, with the zero padding
   reproducing the reference's zero-fill boundary exactly.
 * inverse shift (on gelu(y), C=512): y's channels are produced in 9 groups of
   57 padded to 64 partitions (576 rows = 4.5 blocks; host-permuted fc1_w
   columns), so each group starts at partition 0 or 64 (the HW requires
   compute-engine APs to start at 32-aligned partitions).  The gelu PSUM
   evacuation then writes each group directly into h at its inversely-shifted,
   edge-clipped token positions — the shift costs no extra passes.
   w / the gate / proj all use the same padded-576 channel layout (again via
   host-side weight permutation); padded rows are exactly zero throughout.

The only cross-core coupling is the global (T,H,W) mean feeding the softmax
gate.  It is done as TWO tiny AllReduces (one per frame): the first is
triggered halfway through the kernel and absorbs the cross-core launch skew
under frame-1 compute, so only the second's ~10us floor is exposed.

bf16 matmuls with f32 PSUM accumulation; output f32.  Frame 0's h/w branches
spill to DRAM (bf16) and stream back during the output phase to fit SBUF.
"""

import os
import sys

for _p in ("/opt/trn_rl_repo",):
    if os.path.isdir(_p) and _p not in sys.path:
        sys.path.append(_p)

import numpy as np
import ml_dtypes

import concourse.bass as bass  # noqa: F401
import concourse.mybir as mybir
import concourse.tile as tile
from concourse import bacc
from concourse.bass_utils import run_bass_kernel_spmd

# ---------------------------------------------------------------- constants
SHIFTS = [(1, 1), (1, 0), (1, -1), (0, 1), (0, 0), (0, -1), (-1, 1), (-1, 0), (-1, -1)]
NG = 9
B, T, H, W, C = 2, 8, 56, 56, 512
HID = 1024
NCORES = 8
NF = (B * T) // NCORES          # frames per core = 2
HWTOK = H * W                   # 3136 tokens per frame
RP = W + 1                      # padded row pitch = 57
GUARD = RP + 1                  # 58 zero tokens on each end
FRPAD = RP * H                  # 3192
XHSPAN = GUARD + FRPAD + GUARD  # 3308
RG = 7                          # row groups per frame
RGR = H // RG                   # 8 rows per group
RGT = RGR * W                   # 448 valid tokens per row group
RGP = RGR * RP                  # 456 padded tokens per row group
GS_HID = 114                    # hid shift-group size (9*114 = 1026 >= 1024)
GS_C = 57                       # C shift-group size (9*57 = 513 >= 512)
GPAD = 64                       # C shift groups padded to 64 partitions
CP = NG * GPAD                  # 576 padded C rows
YCB = (CP + 127) // 128         # 5 row-blocks (last half-used)
CCB = C // 128                  # 4
HCB = HID // 128                # 8
MEAN_N = float(T * H * W)

F32 = mybir.dt.float32
BF16 = mybir.dt.bfloat16
BF16_NP = ml_dtypes.bfloat16

_CACHE = {}


def _c_groups():
    """(g, n_ch, real channel range) for the 9 C shift groups."""
    out = []
    for g in range(NG):
        c0 = GS_C * g
        c1 = min(GS_C * (g + 1), C)
        out.append((g, c1 - c0, c0, c1))
    return out


# ---------------------------------------------------------------- device kernel
def build_nc():
    nc = bacc.Bacc("TRN2", target_bir_lowering=False, debug=False, num_devices=NCORES)

    dp = nc.declare_dram_parameter
    xT = dp("xT", [NF, 128, CCB, HWTOK], BF16, isOutput=False)
    fcw = dp("fcw", [128, CCB, NG * 128], BF16, isOutput=False)
    fcb = dp("fcb", [128, NG], F32, isOutput=False)
    fc1w = dp("fc1w", [128, NG, CP], BF16, isOutput=False)
    fc1b = dp("fc1b", [128, YCB], F32, isOutput=False)
    fc2w = dp("fc2w", [128, NG, CP], BF16, isOutput=False)
    fc2b = dp("fc2b", [128, YCB], F32, isOutput=False)
    projw = dp("projw", [128, YCB, C], BF16, isOutput=False)
    projb = dp("projb", [128, C], F32, isOutput=False)
    rw1w = dp("rw1w", [128, YCB, 128], BF16, isOutput=False)
    rw1b = dp("rw1b", [128, 1], F32, isOutput=False)
    rw2w = dp("rw2w", [128, 2 * YCB * 128], BF16, isOutput=False)
    rw2b = dp("rw2b", [128, 2 * YCB], F32, isOutput=False)
    bmask = dp("bmask", [128, B], F32, isOutput=False)
    out_d = dp("out", [NF, HWTOK, C], F32, isOutput=True)

    # spill space for frame 0's h/w branches + collective bounce buffers
    h0d = nc.dram_tensor("h0d", [128, YCB, HWTOK], BF16)
    w0d = nc.dram_tensor("w0d", [128, YCB, HWTOK], BF16)
    ccin = [nc.dram_tensor(f"ccin{f}", [B, 128, YCB], F32) for f in range(NF)]
    ccout = [
        nc.dram_tensor(f"ccout{f}", [B, 128, YCB], F32, addr_space="Shared")
        for f in range(NF)
    ]

    AF = mybir.ActivationFunctionType
    ALU = mybir.AluOpType

    with tile.TileContext(nc, num_cores=NCORES) as tc:
        with (
            tc.tile_pool(name="singles", bufs=1) as singles,
            tc.tile_pool(name="xh_pool", bufs=1) as xh_pool,
            tc.tile_pool(name="h_pool", bufs=1) as h_pool,
            tc.tile_pool(name="w_pool", bufs=1) as w_pool,
            tc.tile_pool(name="xt_pool", bufs=2) as xt_pool,
            tc.tile_pool(name="ostage", bufs=3) as ostage,
            tc.tile_pool(name="dstream", bufs=2) as dstream,
            tc.tile_pool(name="small", bufs=1) as small,
            tc.tile_pool(name="mmpsum", bufs=6, space="PSUM") as mmpsum,
            tc.tile_pool(name="gpsum", bufs=1, space="PSUM") as gpsum,
        ):
            # ---- load weights (resident for the whole kernel)
            def load(name, shape, dtype, src):
                t = singles.tile(shape, dtype, name=name)
                nc.sync.dma_start(out=t, in_=src[:])
                return t

            fcw_s = load("fcw_s", [128, CCB, NG * 128], BF16, fcw)
            fcb_s = load("fcb_s", [128, NG], F32, fcb)
            fc1w_s = load("fc1w_s", [128, NG, CP], BF16, fc1w)
            fc1b_s = load("fc1b_s", [128, YCB], F32, fc1b)
            fc2w_s = load("fc2w_s", [128, NG, CP], BF16, fc2w)
            fc2b_s = load("fc2b_s", [128, YCB], F32, fc2b)
            projw_s = load("projw_s", [128, YCB, C], BF16, projw)
            projb_s = load("projb_s", [128, C], F32, projb)
            rw1w_s = load("rw1w_s", [128, YCB, 128], BF16, rw1w)
            rw1b_s = load("rw1b_s", [128, 1], F32, rw1b)
            rw2w_s = load("rw2w_s", [128, 2 * YCB * 128], BF16, rw2w)
            rw2b_s = load("rw2b_s", [128, 2 * YCB], F32, rw2b)
            bmask_s = load("bmask_s", [128, B], F32, bmask)

            a0_s = singles.tile([128, YCB], F32)   # gate for the h branch

            # xh, padded token layout, persistent across frames.
            xh = xh_pool.tile([128, NG, XHSPAN], BF16)
            # zero guards + per-row pad column once; the body is fully
            # rewritten by every frame's fc pass.
            nc.vector.memset(xh[:, :, :GUARD], 0.0)
            nc.vector.memset(xh[:, :, GUARD + FRPAD:], 0.0)
            xh_rows = xh[:, :, GUARD:GUARD + FRPAD].rearrange(
                "p g (r c) -> p g r c", c=RP
            )
            nc.vector.memset(xh_rows[:, :, :, W:], 0.0)

            hw_tiles = []
            part_sums = []

            for f in range(NF):
                # ---------------- A: xh = gelu(x @ fc_w + fc_b), group-blocked
                for rg in range(RG):
                    xt_t = xt_pool.tile([128, CCB, RGT], BF16, tag="xt")
                    nc.sync.dma_start(
                        out=xt_t, in_=xT[f, :, :, rg * RGT:(rg + 1) * RGT]
                    )
                    for mb in range(NG):
                        ps = mmpsum.tile([128, 512], F32, tag="mm")
                        for k in range(CCB):
                            nc.tensor.matmul(
                                ps[:, :RGT],
                                lhsT=fcw_s[:, k, mb * 128:(mb + 1) * 128],
                                rhs=xt_t[:, k, :],
                                start=(k == 0),
                                stop=(k == CCB - 1),
                            )
                        dst = xh[
                            :, mb, GUARD + rg * RGP:GUARD + (rg + 1) * RGP
                        ].rearrange("p (r c) -> p r c", c=RP)[:, :, :W]
                        src = ps[:, :RGT].rearrange("p (r c) -> p r c", c=W)
                        nc.scalar.activation(
                            out=dst, in_=src, func=AF.Gelu,
                            bias=fcb_s[:, mb:mb + 1],
                        )

                # ---------------- C: h = invshift(gelu(shift(xh) @ fc1_w + b))
                # y channels live in 9 groups of 57 padded to 64 partitions
                # (576 rows = YCB blocks); the inverse shift is applied by the
                # gelu evacuation writing each group at shifted positions.
                h_t = h_pool.tile([128, YCB, HWTOK], BF16, tag="h")
                nc.gpsimd.memset(h_t[:], 0.0)
                h4 = h_t.rearrange("p c (i j) -> p c i j", j=W)
                for rg in range(RG):
                    for mb in range(YCB):
                        M = min(128, CP - mb * 128)
                        ps = mmpsum.tile([128, 512], F32, tag="mm")
                        for g in range(NG):
                            off = -(SHIFTS[g][0] * RP + SHIFTS[g][1])
                            s0 = GUARD + rg * RGP + off
                            nc.tensor.matmul(
                                ps[:M, :RGP],
                                lhsT=fc1w_s[:, g, mb * 128:mb * 128 + M],
                                rhs=xh[:, g, s0:s0 + RGP],
                                start=(g == 0),
                                stop=(g == NG - 1),
                            )
                        ps3 = ps[:, :RGP].rearrange("p (r c) -> p r c", c=RP)
                        # two 64-partition group-halves per block (block 4:
                        # only the lower half carries group 8)
                        for half in range(2):
                            q0 = mb * 128 + half * GPAD
                            g = q0 // GPAD
                            if g >= NG:
                                continue
                            nch = min(GS_C * (g + 1), C) - GS_C * g
                            sh, sw = SHIFTS[g]
                            # h(i',j') = gelu_y(i'+sh, j'+sw); this window
                            # holds gelu_y rows [8rg, 8rg+8)
                            i0 = max(0, 8 * rg - sh)
                            i1 = min(H, 8 * rg + 8 - sh)
                            j0, j1 = max(0, -sw), min(W, W - sw)
                            p0 = half * GPAD
                            nc.scalar.activation(
                                out=h4[p0:p0 + nch, mb, i0:i1, j0:j1],
                                in_=ps3[
                                    p0:p0 + nch,
                                    i0 + sh - 8 * rg:i1 + sh - 8 * rg,
                                    j0 + sw:j1 + sw,
                                ],
                                func=AF.Gelu,
                                bias=fc1b_s[p0:p0 + nch, mb:mb + 1],
                            )

                # ---------------- B: w = gelu(xh @ fc2_w + fc2_b), padded-576
                w_t = w_pool.tile([128, YCB, HWTOK], BF16, tag="w")
                # zero the unused upper half of the last row-block so the
                # gate reduction sees exact zeros there
                nc.vector.memset(w_t[GPAD:, YCB - 1, :], 0.0)
                for rg in range(RG):
                    for mb in range(YCB):
                        M = min(128, CP - mb * 128)
                        ps = mmpsum.tile([128, 512], F32, tag="mm")
                        for g in range(NG):
                            s0 = GUARD + rg * RGP
                            nc.tensor.matmul(
                                ps[:M, :RGP],
                                lhsT=fc2w_s[:, g, mb * 128:mb * 128 + M],
                                rhs=xh[:, g, s0:s0 + RGP],
                                start=(g == 0),
                                stop=(g == NG - 1),
                            )
                        dst = w_t[:M, mb, rg * RGT:(rg + 1) * RGT].rearrange(
                            "p (r c) -> p r c", c=W
                        )
                        srcp = ps[:M, :RGP].rearrange("p (r c) -> p r c", c=RP)[:, :, :W]
                        nc.scalar.activation(
                            out=dst, in_=srcp, func=AF.Gelu,
                            bias=fc2b_s[:M, mb:mb + 1],
                        )

                # ---------------- per-frame gate partial sum + AllReduce
                hs = small.tile([128, YCB], F32, tag=f"hs{f}")
                nc.vector.tensor_reduce(
                    out=hs, in_=h_t[:], axis=mybir.AxisListType.X, op=ALU.add
                )
                ws = small.tile([128, YCB], F32, tag=f"ws{f}")
                nc.vector.tensor_reduce(
                    out=ws, in_=w_t[:], axis=mybir.AxisListType.X, op=ALU.add
                )
                part = small.tile([128, YCB], F32, tag=f"part{f}")
                nc.vector.tensor_tensor(part, hs, ws, ALU.add)
                part_sums.append(part)
                # mask into the own-batch row and AllReduce; frame 0's
                # collective overlaps frame 1's compute (and absorbs the
                # cross-core launch skew).
                t0 = small.tile([128, YCB], F32, tag=f"cca{f}")
                nc.vector.tensor_scalar_mul(t0, part, bmask_s[:, 0:1])
                t1 = small.tile([128, YCB], F32, tag=f"ccb{f}")
                nc.vector.tensor_scalar_mul(t1, part, bmask_s[:, 1:2])
                nc.sync.dma_start(out=ccin[f][0], in_=t0)
                nc.sync.dma_start(out=ccin[f][1], in_=t1)
                nc.gpsimd.collective_compute(
                    "AllReduce",
                    ALU.add,
                    replica_groups=[list(range(NCORES))],
                    ins=[ccin[f][:]],
                    outs=[ccout[f][:]],
                )

                if f == 0:
                    # spill frame 0's branches; streamed back during D0
                    nc.sync.dma_start(out=h0d[:], in_=h_t[:])
                    nc.sync.dma_start(out=w0d[:], in_=w_t[:])
                hw_tiles.append((h_t, w_t))

            # ---------------- combine the two AllReduce results -> z
            acc = []
            for f in range(NF):
                za = small.tile([128, YCB], F32, tag=f"za{f}")
                nc.sync.dma_start(out=za, in_=ccout[f][0])
                zb = small.tile([128, YCB], F32, tag=f"zb{f}")
                nc.sync.dma_start(out=zb, in_=ccout[f][1])
                nc.vector.tensor_scalar_mul(za, za, bmask_s[:, 0:1])
                nc.vector.tensor_scalar_mul(zb, zb, bmask_s[:, 1:2])
                s = small.tile([128, YCB], F32, tag=f"zs{f}")
                nc.vector.tensor_tensor(s, za, zb, ALU.add)
                acc.append(s)
            zsum = small.tile([128, YCB], F32, tag="zsum")
            nc.vector.tensor_tensor(zsum, acc[0], acc[1], ALU.add)
            zbf = small.tile([128, YCB], BF16, tag="zbf")
            nc.vector.tensor_copy(out=zbf, in_=zsum)

            # ---------------- gate: a = softmax over the 2 streams
            # (1/MEAN_N is folded into rw1w on the host)
            psg = gpsum.tile([128, 1], F32, tag="psg")
            for k in range(YCB):
                nc.tensor.matmul(
                    psg,
                    lhsT=rw1w_s[:, k, :],
                    rhs=zbf[:, k:k + 1],
                    start=(k == 0),
                    stop=(k == YCB - 1),
                )
            gv = small.tile([128, 1], BF16, tag="gv")
            nc.scalar.activation(out=gv, in_=psg, func=AF.Gelu, bias=rw1b_s[:, 0:1])
            psu = gpsum.tile([128, 2 * YCB], F32, tag="psu")
            for m in range(2 * YCB):
                nc.tensor.matmul(
                    psu[:, m:m + 1],
                    lhsT=rw2w_s[:, m * 128:(m + 1) * 128],
                    rhs=gv,
                    start=True,
                    stop=True,
                )
            uv = small.tile([128, 2 * YCB], F32, tag="uv")
            nc.vector.tensor_tensor(uv, psu, rw2b_s, ALU.add)
            l0, l1 = uv[:, 0:YCB], uv[:, YCB:2 * YCB]
            mx = small.tile([128, YCB], F32, tag="mx")
            nc.vector.tensor_tensor(mx, l0, l1, ALU.max)
            d0 = small.tile([128, YCB], F32, tag="d0")
            nc.vector.tensor_tensor(d0, l0, mx, ALU.subtract)
            d1 = small.tile([128, YCB], F32, tag="d1")
            nc.vector.tensor_tensor(d1, l1, mx, ALU.subtract)
            e0 = small.tile([128, YCB], F32, tag="e0")
            nc.scalar.activation(out=e0, in_=d0, func=AF.Exp)
            e1 = small.tile([128, YCB], F32, tag="e1")
            nc.scalar.activation(out=e1, in_=d1, func=AF.Exp)
            esum = small.tile([128, YCB], F32, tag="esum")
            nc.vector.tensor_tensor(esum, e0, e1, ALU.add)
            rec = small.tile([128, YCB], F32, tag="rec")
            nc.vector.reciprocal(rec, esum)
            nc.vector.tensor_tensor(a0_s, e0, rec, ALU.mult)

            # ---------------- D: out = (a0*h + (1-a0)*w) @ proj_w + proj_b
            def proj_blocks(gated_ap, fidx, tbase, ntok):
                """gated_ap: [128, YCB, ntok] bf16 SBUF ap (padded-576)."""
                m0 = 0
                while m0 < ntok:
                    M = min(128, ntok - m0)
                    pp = mmpsum.tile([128, 512], F32, tag="mm")
                    for kb in range(YCB):
                        nc.tensor.matmul(
                            pp[:M, :C],
                            lhsT=gated_ap[:, kb, m0:m0 + M],
                            rhs=projw_s[:, kb, :],
                            start=(kb == 0),
                            stop=(kb == YCB - 1),
                        )
                    ot = ostage.tile([128, C], F32, tag="ot")
                    nc.vector.tensor_tensor(ot[:M], pp[:M, :C], projb_s[:M], ALU.add)
                    nc.sync.dma_start(
                        out=out_d[fidx, tbase + m0:tbase + m0 + M, :], in_=ot[:M]
                    )
                    m0 += M

            def gate_inplace(h_ap, w_ap):
                """h_ap <- a0*h + (1-a0)*w   (= w + a0*(h-w)), in place."""
                nc.vector.tensor_tensor(h_ap, h_ap, w_ap, ALU.subtract)
                for kb in range(YCB):
                    nc.scalar.activation(
                        out=h_ap[:, kb, :], in_=h_ap[:, kb, :],
                        func=AF.Copy, scale=a0_s[:, kb:kb + 1],
                    )
                nc.vector.tensor_tensor(h_ap, h_ap, w_ap, ALU.add)

            # frame 1 from SBUF
            h1, w1 = hw_tiles[1]
            gate_inplace(h1[:], w1[:])
            proj_blocks(h1, 1, 0, HWTOK)

            # frame 0 streamed back from DRAM in 512-token chunks
            ck0 = 0
            while ck0 < HWTOK:
                CK = min(512, HWTOK - ck0)
                hc = dstream.tile([128, YCB, 512], BF16, tag="hc")
                nc.sync.dma_start(out=hc[:, :, :CK], in_=h0d[:, :, ck0:ck0 + CK])
                wc = dstream.tile([128, YCB, 512], BF16, tag="wc")
                nc.sync.dma_start(out=wc[:, :, :CK], in_=w0d[:, :, ck0:ck0 + CK])
                gate_inplace(hc[:, :, :CK], wc[:, :, :CK])
                proj_blocks(hc, 0, ck0, CK)
                ck0 += CK

    nc.compile()
    return nc


# ---------------------------------------------------------------- host side
def _prep_weights(fc_w, fc_b, fc1_w, fc1_b, fc2_w, fc2_b,
                  rw1_w, rw1_b, rw2_w, rw2_b, proj_w, proj_b):
    f32 = np.float32

    # padded-576 C layout: padded row q = 64*g + s  <->  channel c = 57*g + s
    qof = np.full((CP,), -1, np.int64)
    for g, nch, c0, _ in _c_groups():
        qof[GPAD * g:GPAD * g + nch] = np.arange(c0, c0 + nch)
    qvalid = qof >= 0
    qidx = np.where(qvalid, np.maximum(qof, 0), 0)

    def cols_to_padded576(m):  # [R, C] -> [R, CP] with zero pad cols
        out = np.zeros((m.shape[0], CP), f32)
        out[:, qvalid] = m[:, qidx[qvalid]]
        return out

    def rows_to_padded576(m):  # [C, N] -> [CP, N] with zero pad rows
        out = np.zeros((CP, m.shape[1]), f32)
        out[qvalid] = m[qidx[qvalid]]
        return out

    def vec_to_padded576(v):
        out = np.zeros((CP,), f32)
        out[qvalid] = v[qidx[qvalid]]
        return out

    # fc: columns permuted into 9 HID-groups of 114 (112 for g=8), pad to 128
    fcwp = np.zeros((C, NG * 128), f32)
    fcbp = np.zeros((NG * 128,), f32)
    for g in range(NG):
        n = min(GS_HID * (g + 1), HID) - GS_HID * g
        fcwp[:, 128 * g:128 * g + n] = fc_w[:, GS_HID * g:GS_HID * g + n]
        fcbp[128 * g:128 * g + n] = fc_b[GS_HID * g:GS_HID * g + n]
    fcw_h = np.ascontiguousarray(
        fcwp.reshape(CCB, 128, NG * 128).transpose(1, 0, 2)
    ).astype(BF16_NP)
    fcb_h = np.ascontiguousarray(fcbp.reshape(NG, 128).T).astype(f32)

    def hid_rows_grouped(wm):  # [HID, CP] -> [128, NG, CP] padded group rows
        wp = np.zeros((NG * 128, wm.shape[1]), f32)
        for g in range(NG):
            n = min(GS_HID * (g + 1), HID) - GS_HID * g
            wp[128 * g:128 * g + n] = wm[GS_HID * g:GS_HID * g + n]
        return np.ascontiguousarray(
            wp.reshape(NG, 128, wm.shape[1]).transpose(1, 0, 2)
        ).astype(BF16_NP)

    fc1w_h = hid_rows_grouped(cols_to_padded576(fc1_w))
    fc2w_h = hid_rows_grouped(cols_to_padded576(fc2_w))

    fc1bp = vec_to_padded576(fc1_b)
    fc2bp = vec_to_padded576(fc2_b)
    padb = np.zeros((YCB * 128,), f32)
    fc1b_h = padb.copy(); fc1b_h[:CP] = fc1bp
    fc1b_h = np.ascontiguousarray(fc1b_h.reshape(YCB, 128).T).astype(f32)
    fc2b_h = padb.copy(); fc2b_h[:CP] = fc2bp
    fc2b_h = np.ascontiguousarray(fc2b_h.reshape(YCB, 128).T).astype(f32)

    # proj: rows in padded-576 layout (pad rows zero), cols plain C
    projwp = np.zeros((YCB * 128, C), f32)
    projwp[:CP] = rows_to_padded576(proj_w)
    projw_h = np.ascontiguousarray(
        projwp.reshape(YCB, 128, C).transpose(1, 0, 2)
    ).astype(BF16_NP)
    projb_h = np.ascontiguousarray(
        np.broadcast_to(proj_b[None, :], (128, C))
    ).astype(f32)

    # rw1: rows in padded-576 layout, scaled by 1/MEAN_N (folds the mean)
    rw1p = np.zeros((YCB * 128, C // 4), f32)
    rw1p[:CP] = rows_to_padded576(rw1_w / MEAN_N)
    rw1w_h = np.ascontiguousarray(
        rw1p.reshape(YCB, 128, C // 4).transpose(1, 0, 2)
    ).astype(BF16_NP)
    rw1b_h = np.ascontiguousarray(rw1_b[:, None]).astype(f32)

    # rw2 columns: stream-0 logits in padded cols [0, CP), stream-1 logits in
    # padded cols [YCB*128, YCB*128 + CP) — so the device's 128-wide M-blocks
    # 0..4 are stream 0 and 5..9 are stream 1.
    NQ = YCB * 128
    rw2p = np.zeros((128, 2 * NQ), f32)
    rw2p[:, 0:CP][:, qvalid] = rw2_w[:, 2 * qidx[qvalid]]
    rw2p[:, NQ:NQ + CP][:, qvalid] = rw2_w[:, 2 * qidx[qvalid] + 1]
    rw2w_h = np.ascontiguousarray(rw2p).astype(BF16_NP)
    rw2b_full = np.zeros((2 * NQ,), f32)
    rw2b_full[0:CP][qvalid] = rw2_b[2 * qidx[qvalid]]
    rw2b_full[NQ:NQ + CP][qvalid] = rw2_b[2 * qidx[qvalid] + 1]
    rw2b_h = np.ascontiguousarray(rw2b_full.reshape(2 * YCB, 128).T).astype(f32)

    return dict(
        fcw=fcw_h, fcb=fcb_h, fc1w=fc1w_h, fc1b=fc1b_h, fc2w=fc2w_h,
        fc2b=fc2b_h, projw=projw_h, projb=projb_h, rw1w=rw1w_h, rw1b=rw1b_h,
        rw2w=rw2w_h, rw2b=rw2b_h,
    )


def _get_nc():
    if "nc" not in _CACHE:
        _CACHE["nc"] = build_nc()
    return _CACHE["nc"]


def run(inputs, trace=False, trace_kwargs=None):
    """Run the SPMD kernel; returns (full_output, BassKernelResults)."""
    x = np.asarray(inputs["x"], np.float32)
    shared = _prep_weights(
        np.asarray(inputs["fc_w"], np.float32), np.asarray(inputs["fc_b"], np.float32),
        np.asarray(inputs["fc1_w"], np.float32), np.asarray(inputs["fc1_b"], np.float32),
        np.asarray(inputs["fc2_w"], np.float32), np.asarray(inputs["fc2_b"], np.float32),
        np.asarray(inputs["rw1_w"], np.float32), np.asarray(inputs["rw1_b"], np.float32),
        np.asarray(inputs["rw2_w"], np.float32), np.asarray(inputs["rw2_b"], np.float32),
        np.asarray(inputs["proj_w"], np.float32), np.asarray(inputs["proj_b"], np.float32),
    )

    xf = x.reshape(B * T, HWTOK, C)
    in_maps = []
    for c in range(NCORES):
        sh = xf[NF * c:NF * (c + 1)]                      # [NF, 3136, 512]
        xt = sh.transpose(0, 2, 1).reshape(NF, CCB, 128, HWTOK)
        xt = np.ascontiguousarray(xt.transpose(0, 2, 1, 3)).astype(BF16_NP)
        bm = np.zeros((128, B), np.float32)
        bm[:, (NF * c) // T] = 1.0
        m = dict(shared)
        m["xT"] = xt
        m["bmask"] = bm
        in_maps.append(m)

    nc = _get_nc()
    res = run_bass_kernel_spmd(
        nc, in_maps, list(range(NCORES)),
        trace=trace, **(dict(trace_kwargs=trace_kwargs) if trace_kwargs else {}),
    )

    out = np.empty((B * T, HWTOK, C), np.float32)
    for c in range(NCORES):
        out[NF * c:NF * (c + 1)] = res.results[c]["out"]
    return out.reshape(B, T, H, W, C), res


def kernel(**inputs) -> np.ndarray:
    full, _ = run(inputs, trace=False)
    return full


# revision 9
# speedup vs baseline: 1.2330x; 1.0863x over previous
"""Trainium2 Bass kernel for nn_Mlp_cnn_shift (dense CNN MLP with 3x3 patch-shift
and a softmax-gated mix of two branches).

Strategy
--------
Data-parallel over the 16 (B,T) frames: each of the 8 NeuronCores processes 2
frames end-to-end.  All activations are kept channel-major ([C, tokens]) so the
channel contraction of every matmul has K on partitions, and `x` is
pre-transposed/cast on the host so no on-device transpose is needed.

Patch-shift handling:
 * forward shift (on xh, HID=1024): xh is stored in a zero-padded token layout
   (row pitch 57 = 56 cols + 1 zero pad col, 58-token zero guards per frame)
   and in 9 channel groups of 114 padded to 128 partitions each (host-permuted
   fc_w columns / fc1_w+fc2_w rows).  Every (dh,dw) roll then becomes a pure
   token offset in the fc1 matmul's rhs access pattern, with the zero padding
   reproducing the reference's zero-fill boundary exactly.
 * inverse shift (on gelu(y), C=512): y's channels are produced in 9 groups of
   57 padded to 64 partitions (576 rows = 4.5 blocks; host-permuted fc1_w
   columns), so each group starts at partition 0 or 64 (the HW requires
   compute-engine APs to start at 32-aligned partitions).  The gelu PSUM
   evacuation then writes each group directly into h at its inversely-shifted,
   edge-clipped token positions — the shift costs no extra passes.
   w / the gate / proj all use the same padded-576 channel layout (again via
   host-side weight permutation); padded rows are exactly zero throughout.

The only cross-core coupling is the global (T,H,W) mean feeding the softmax
gate.  It is done as TWO tiny AllReduces (one per frame): the first is
triggered halfway through the kernel and absorbs the cross-core launch skew
under frame-1 compute, so only the second's ~10us floor is exposed.

bf16 matmuls with f32 PSUM accumulation; output f32.  Frame 0's h/w branches
spill to DRAM (bf16) and stream back during the output phase to fit SBUF.
"""

import os
import sys

for _p in ("/opt/trn_rl_repo",):
    if os.path.isdir(_p) and _p not in sys.path:
        sys.path.append(_p)

import numpy as np
import ml_dtypes

import concourse.bass as bass  # noqa: F401
import concourse.mybir as mybir
import concourse.tile as tile
from concourse import bacc
from concourse.bass_utils import run_bass_kernel_spmd

# ---------------------------------------------------------------- constants
SHIFTS = [(1, 1), (1, 0), (1, -1), (0, 1), (0, 0), (0, -1), (-1, 1), (-1, 0), (-1, -1)]
NG = 9
B, T, H, W, C = 2, 8, 56, 56, 512
HID = 1024
NCORES = 8
NF = (B * T) // NCORES          # frames per core = 2
HWTOK = H * W                   # 3136 tokens per frame
RP = W + 1                      # padded row pitch = 57
GUARD = RP + 1                  # 58 zero tokens on each end
FRPAD = RP * H                  # 3192
XHSPAN = GUARD + FRPAD + GUARD  # 3308
RG = 7                          # row groups per frame
RGR = H // RG                   # 8 rows per group
RGT = RGR * W                   # 448 valid tokens per row group
RGP = RGR * RP                  # 456 padded tokens per row group
GS_HID = 114                    # hid shift-group size (9*114 = 1026 >= 1024)
GS_C = 57                       # C shift-group size (9*57 = 513 >= 512)
GPAD = 64                       # C shift groups padded to 64 partitions
CP = NG * GPAD                  # 576 padded C rows
YCB = (CP + 127) // 128         # 5 row-blocks (last half-used)
CCB = C // 128                  # 4
HCB = HID // 128                # 8
MEAN_N = float(T * H * W)

F32 = mybir.dt.float32
BF16 = mybir.dt.bfloat16
BF16_NP = ml_dtypes.bfloat16

_CACHE = {}


def _c_groups():
    """(g, n_ch, real channel range) for the 9 C shift groups."""
    out = []
    for g in range(NG):
        c0 = GS_C * g
        c1 = min(GS_C * (g + 1), C)
        out.append((g, c1 - c0, c0, c1))
    return out


# ---------------------------------------------------------------- device kernel
def build_nc():
    nc = bacc.Bacc("TRN2", target_bir_lowering=False, debug=False, num_devices=NCORES)

    dp = nc.declare_dram_parameter
    xT = dp("xT", [NF, 128, CCB, HWTOK], BF16, isOutput=False)
    fcw = dp("fcw", [128, CCB, NG * 128], BF16, isOutput=False)
    fcb = dp("fcb", [128, NG], F32, isOutput=False)
    fc1w = dp("fc1w", [128, NG, CP], BF16, isOutput=False)
    fc1b = dp("fc1b", [128, YCB], F32, isOutput=False)
    fc2w = dp("fc2w", [128, NG, CP], BF16, isOutput=False)
    fc2b = dp("fc2b", [128, YCB], F32, isOutput=False)
    projw = dp("projw", [128, YCB, C], BF16, isOutput=False)
    projb = dp("projb", [128, C], F32, isOutput=False)
    rw1w = dp("rw1w", [128, YCB, 128], BF16, isOutput=False)
    rw1b = dp("rw1b", [128, 1], F32, isOutput=False)
    rw2w = dp("rw2w", [128, 2 * YCB * 128], BF16, isOutput=False)
    rw2b = dp("rw2b", [128, 2 * YCB], F32, isOutput=False)
    bmask = dp("bmask", [128, B], F32, isOutput=False)
    out_d = dp("out", [NF, HWTOK, C], F32, isOutput=True)

    # spill space for the w branch of each frame + collective bounce buffers
    wsp = [nc.dram_tensor(f"wsp{f}", [128, YCB, HWTOK], BF16) for f in range(NF)]
    ccin = [nc.dram_tensor(f"ccin{f}", [B, 128, YCB], F32) for f in range(NF)]
    ccout = [
        nc.dram_tensor(f"ccout{f}", [B, 128, YCB], F32, addr_space="Shared")
        for f in range(NF)
    ]

    AF = mybir.ActivationFunctionType
    ALU = mybir.AluOpType

    with tile.TileContext(nc, num_cores=NCORES) as tc:
        with (
            tc.tile_pool(name="singles", bufs=1) as singles,
            tc.tile_pool(name="xh_pool", bufs=1) as xh_pool,
            tc.tile_pool(name="h_pool", bufs=2) as h_pool,
            tc.tile_pool(name="w_pool", bufs=2) as w_pool,
            tc.tile_pool(name="xt_pool", bufs=2) as xt_pool,
            tc.tile_pool(name="ostage", bufs=3) as ostage,
            tc.tile_pool(name="dstream", bufs=2) as dstream,
            tc.tile_pool(name="small", bufs=1) as small,
            tc.tile_pool(name="mmpsum", bufs=6, space="PSUM") as mmpsum,
            tc.tile_pool(name="gpsum", bufs=1, space="PSUM") as gpsum,
        ):
            # ---- load weights (resident for the whole kernel)
            def load(name, shape, dtype, src):
                t = singles.tile(shape, dtype, name=name)
                nc.sync.dma_start(out=t, in_=src[:])
                return t

            fcw_s = load("fcw_s", [128, CCB, NG * 128], BF16, fcw)
            fcb_s = load("fcb_s", [128, NG], F32, fcb)
            fc1w_s = load("fc1w_s", [128, NG, CP], BF16, fc1w)
            fc1b_s = load("fc1b_s", [128, YCB], F32, fc1b)
            fc2w_s = load("fc2w_s", [128, NG, CP], BF16, fc2w)
            fc2b_s = load("fc2b_s", [128, YCB], F32, fc2b)
            projw_s = load("projw_s", [128, YCB, C], BF16, projw)
            projb_s = load("projb_s", [128, C], F32, projb)
            rw1w_s = load("rw1w_s", [128, YCB, 128], BF16, rw1w)
            rw1b_s = load("rw1b_s", [128, 1], F32, rw1b)
            rw2w_s = load("rw2w_s", [128, 2 * YCB * 128], BF16, rw2w)
            rw2b_s = load("rw2b_s", [128, 2 * YCB], F32, rw2b)
            bmask_s = load("bmask_s", [128, B], F32, bmask)

            a0_s = singles.tile([128, YCB], F32)   # gate for the h branch

            # xh, padded token layout, persistent across frames.
            xh = xh_pool.tile([128, NG, XHSPAN], BF16)
            # zero guards + per-row pad column once; the body is fully
            # rewritten by every frame's fc pass.
            nc.vector.memset(xh[:, :, :GUARD], 0.0)
            nc.vector.memset(xh[:, :, GUARD + FRPAD:], 0.0)
            xh_rows = xh[:, :, GUARD:GUARD + FRPAD].rearrange(
                "p g (r c) -> p g r c", c=RP
            )
            nc.vector.memset(xh_rows[:, :, :, W:], 0.0)

            hw_tiles = []
            part_sums = []

            for f in range(NF):
                # ---------------- A: xh = gelu(x @ fc_w + fc_b), group-blocked
                for rg in range(RG):
                    xt_t = xt_pool.tile([128, CCB, RGT], BF16, tag="xt")
                    nc.sync.dma_start(
                        out=xt_t, in_=xT[f, :, :, rg * RGT:(rg + 1) * RGT]
                    )
                    for mb in range(NG):
                        ps = mmpsum.tile([128, 512], F32, tag="mm")
                        for k in range(CCB):
                            nc.tensor.matmul(
                                ps[:, :RGT],
                                lhsT=fcw_s[:, k, mb * 128:(mb + 1) * 128],
                                rhs=xt_t[:, k, :],
                                start=(k == 0),
                                stop=(k == CCB - 1),
                            )
                        dst = xh[
                            :, mb, GUARD + rg * RGP:GUARD + (rg + 1) * RGP
                        ].rearrange("p (r c) -> p r c", c=RP)[:, :, :W]
                        src = ps[:, :RGT].rearrange("p (r c) -> p r c", c=W)
                        nc.scalar.activation(
                            out=dst, in_=src, func=AF.Gelu,
                            bias=fcb_s[:, mb:mb + 1],
                        )

                # ---------------- C: h = invshift(gelu(shift(xh) @ fc1_w + b))
                # y channels live in 9 groups of 57 padded to 64 partitions
                # (576 rows = YCB blocks); the inverse shift is applied by the
                # gelu evacuation writing each group at shifted positions.
                h_t = h_pool.tile([128, YCB, HWTOK], BF16, tag="h")
                nc.gpsimd.memset(h_t[:], 0.0)
                h4 = h_t.rearrange("p c (i j) -> p c i j", j=W)
                hsum_st = small.tile([128, YCB, RG], F32, tag=f"hsst{f}")
                nc.vector.memset(hsum_st[:], 0.0)
                wsum_st = small.tile([128, YCB, RG], F32, tag=f"wsst{f}")
                nc.vector.memset(wsum_st[:], 0.0)
                for rg in range(RG):
                    for mb in range(YCB):
                        M = min(128, CP - mb * 128)
                        ps = mmpsum.tile([128, 512], F32, tag="mm")
                        for g in range(NG):
                            off = -(SHIFTS[g][0] * RP + SHIFTS[g][1])
                            s0 = GUARD + rg * RGP + off
                            nc.tensor.matmul(
                                ps[:M, :RGP],
                                lhsT=fc1w_s[:, g, mb * 128:mb * 128 + M],
                                rhs=xh[:, g, s0:s0 + RGP],
                                start=(g == 0),
                                stop=(g == NG - 1),
                            )
                        ps3 = ps[:, :RGP].rearrange("p (r c) -> p r c", c=RP)
                        # two 64-partition group-halves per block (block 4:
                        # only the lower half carries group 8)
                        for half in range(2):
                            q0 = mb * 128 + half * GPAD
                            g = q0 // GPAD
                            if g >= NG:
                                continue
                            nch = min(GS_C * (g + 1), C) - GS_C * g
                            sh, sw = SHIFTS[g]
                            # h(i',j') = gelu_y(i'+sh, j'+sw); this window
                            # holds gelu_y rows [8rg, 8rg+8)
                            i0 = max(0, 8 * rg - sh)
                            i1 = min(H, 8 * rg + 8 - sh)
                            j0, j1 = max(0, -sw), min(W, W - sw)
                            p0 = half * GPAD
                            nc.scalar.activation(
                                out=h4[p0:p0 + nch, mb, i0:i1, j0:j1],
                                in_=ps3[
                                    p0:p0 + nch,
                                    i0 + sh - 8 * rg:i1 + sh - 8 * rg,
                                    j0 + sw:j1 + sw,
                                ],
                                func=AF.Gelu,
                                bias=fc1b_s[p0:p0 + nch, mb:mb + 1],
                                accum_out=hsum_st[p0:p0 + nch, mb, rg:rg + 1],
                            )

                # ---------------- B: w = gelu(xh @ fc2_w + fc2_b), padded-576,
                # built per row-group and spilled to DRAM (streamed back in D)
                for rg in range(RG):
                    w_rg = w_pool.tile([128, YCB, RGT], BF16, tag="wrg")
                    if True:
                        nc.vector.memset(w_rg[GPAD:, YCB - 1, :], 0.0)
                    for mb in range(YCB):
                        M = min(128, CP - mb * 128)
                        ps = mmpsum.tile([128, 512], F32, tag="mm")
                        for g in range(NG):
                            s0 = GUARD + rg * RGP
                            nc.tensor.matmul(
                                ps[:M, :RGP],
                                lhsT=fc2w_s[:, g, mb * 128:mb * 128 + M],
                                rhs=xh[:, g, s0:s0 + RGP],
                                start=(g == 0),
                                stop=(g == NG - 1),
                            )
                        dst = w_rg[:M, mb, :].rearrange("p (r c) -> p r c", c=W)
                        srcp = ps[:M, :RGP].rearrange("p (r c) -> p r c", c=RP)[:, :, :W]
                        nc.scalar.activation(
                            out=dst, in_=srcp, func=AF.Gelu,
                            bias=fc2b_s[:M, mb:mb + 1],
                            accum_out=wsum_st[:M, mb, rg:rg + 1],
                        )
                    nc.sync.dma_start(
                        out=wsp[f][:, :, rg * RGT:(rg + 1) * RGT], in_=w_rg[:]
                    )

                # ---------------- per-frame gate partial sum + AllReduce
                hs = small.tile([128, YCB], F32, tag=f"hs{f}")
                nc.vector.tensor_reduce(
                    out=hs, in_=hsum_st[:], axis=mybir.AxisListType.X, op=ALU.add
                )
                ws = small.tile([128, YCB], F32, tag=f"ws{f}")
                nc.vector.tensor_reduce(
                    out=ws, in_=wsum_st[:], axis=mybir.AxisListType.X, op=ALU.add
                )
                part = small.tile([128, YCB], F32, tag=f"part{f}")
                nc.vector.tensor_tensor(part, hs, ws, ALU.add)
                part_sums.append(part)
                # mask into the own-batch row and AllReduce; frame 0's
                # collective overlaps frame 1's compute (and absorbs the
                # cross-core launch skew).
                t0 = small.tile([128, YCB], F32, tag=f"cca{f}")
                nc.vector.tensor_scalar_mul(t0, part, bmask_s[:, 0:1])
                t1 = small.tile([128, YCB], F32, tag=f"ccb{f}")
                nc.vector.tensor_scalar_mul(t1, part, bmask_s[:, 1:2])
                nc.sync.dma_start(out=ccin[f][0], in_=t0)
                nc.sync.dma_start(out=ccin[f][1], in_=t1)
                nc.gpsimd.collective_compute(
                    "AllReduce",
                    ALU.add,
                    replica_groups=[list(range(NCORES))],
                    ins=[ccin[f][:]],
                    outs=[ccout[f][:]],
                )

                hw_tiles.append(h_t)

            # keep TensorE's activity monitor warm across the second
            # AllReduce's latency window (junk matmuls, results unread) —
            # otherwise the whole output phase runs at the 4/8 cold clock
            for wi in range(120):
                wp = mmpsum.tile([128, 512], F32, tag="mm", name=f"warm{wi}")
                nc.tensor.matmul(
                    wp[:, :512],
                    lhsT=fcw_s[:, 0, 0:128],
                    rhs=fcw_s[:, 1, 0:512],
                    start=True,
                    stop=True,
                )

            # ---------------- combine the two AllReduce results -> z
            acc = []
            for f in range(NF):
                za = small.tile([128, YCB], F32, tag=f"za{f}")
                nc.sync.dma_start(out=za, in_=ccout[f][0])
                zb = small.tile([128, YCB], F32, tag=f"zb{f}")
                nc.sync.dma_start(out=zb, in_=ccout[f][1])
                nc.vector.tensor_scalar_mul(za, za, bmask_s[:, 0:1])
                nc.vector.tensor_scalar_mul(zb, zb, bmask_s[:, 1:2])
                s = small.tile([128, YCB], F32, tag=f"zs{f}")
                nc.vector.tensor_tensor(s, za, zb, ALU.add)
                acc.append(s)
            zsum = small.tile([128, YCB], F32, tag="zsum")
            nc.vector.tensor_tensor(zsum, acc[0], acc[1], ALU.add)
            zbf = small.tile([128, YCB], BF16, tag="zbf")
            nc.vector.tensor_copy(out=zbf, in_=zsum)

            # ---------------- gate: a = softmax over the 2 streams
            # (1/MEAN_N is folded into rw1w on the host)
            psg = gpsum.tile([128, 1], F32, tag="psg")
            for k in range(YCB):
                nc.tensor.matmul(
                    psg,
                    lhsT=rw1w_s[:, k, :],
                    rhs=zbf[:, k:k + 1],
                    start=(k == 0),
                    stop=(k == YCB - 1),
                )
            gv = small.tile([128, 1], BF16, tag="gv")
            nc.scalar.activation(out=gv, in_=psg, func=AF.Gelu, bias=rw1b_s[:, 0:1])
            psu = gpsum.tile([128, 2 * YCB], F32, tag="psu")
            for m in range(2 * YCB):
                nc.tensor.matmul(
                    psu[:, m:m + 1],
                    lhsT=rw2w_s[:, m * 128:(m + 1) * 128],
                    rhs=gv,
                    start=True,
                    stop=True,
                )
            uv = small.tile([128, 2 * YCB], F32, tag="uv")
            nc.vector.tensor_tensor(uv, psu, rw2b_s, ALU.add)
            l0, l1 = uv[:, 0:YCB], uv[:, YCB:2 * YCB]
            mx = small.tile([128, YCB], F32, tag="mx")
            nc.vector.tensor_tensor(mx, l0, l1, ALU.max)
            d0 = small.tile([128, YCB], F32, tag="d0")
            nc.vector.tensor_tensor(d0, l0, mx, ALU.subtract)
            d1 = small.tile([128, YCB], F32, tag="d1")
            nc.vector.tensor_tensor(d1, l1, mx, ALU.subtract)
            e0 = small.tile([128, YCB], F32, tag="e0")
            nc.scalar.activation(out=e0, in_=d0, func=AF.Exp)
            e1 = small.tile([128, YCB], F32, tag="e1")
            nc.scalar.activation(out=e1, in_=d1, func=AF.Exp)
            esum = small.tile([128, YCB], F32, tag="esum")
            nc.vector.tensor_tensor(esum, e0, e1, ALU.add)
            rec = small.tile([128, YCB], F32, tag="rec")
            nc.vector.reciprocal(rec, esum)
            nc.vector.tensor_tensor(a0_s, e0, rec, ALU.mult)

            # ---------------- D: out = (a0*h + (1-a0)*w) @ proj_w + proj_b
            def proj_blocks(gated_ap, fidx, tbase, ntok):
                """gated_ap: [128, YCB, ntok] bf16 SBUF ap (padded-576)."""
                m0 = 0
                while m0 < ntok:
                    M = min(128, ntok - m0)
                    pp = mmpsum.tile([128, 512], F32, tag="mm")
                    for kb in range(YCB):
                        nc.tensor.matmul(
                            pp[:M, :C],
                            lhsT=gated_ap[:, kb, m0:m0 + M],
                            rhs=projw_s[:, kb, :],
                            start=(kb == 0),
                            stop=(kb == YCB - 1),
                        )
                    ot = ostage.tile([128, C], F32, tag="ot")
                    nc.vector.tensor_tensor(ot[:M], pp[:M, :C], projb_s[:M], ALU.add)
                    nc.sync.dma_start(
                        out=out_d[fidx, tbase + m0:tbase + m0 + M, :], in_=ot[:M]
                    )
                    m0 += M

            def gate_inplace(h_ap, w_ap):
                """h_ap <- a0*h + (1-a0)*w   (= w + a0*(h-w)), in place."""
                nc.vector.tensor_tensor(h_ap, h_ap, w_ap, ALU.subtract)
                for kb in range(YCB):
                    nc.scalar.activation(
                        out=h_ap[:, kb, :], in_=h_ap[:, kb, :],
                        func=AF.Copy, scale=a0_s[:, kb:kb + 1],
                    )
                nc.vector.tensor_tensor(h_ap, h_ap, w_ap, ALU.add)

            # h is resident for both frames; stream each frame's w back in
            # 512-token chunks, gate in place on the h slice, then project.
            for fidx in (1, 0):
                h_t = hw_tiles[fidx]
                ck0 = 0
                while ck0 < HWTOK:
                    CK = min(512, HWTOK - ck0)
                    wc = dstream.tile([128, YCB, 512], BF16, tag="wc")
                    nc.sync.dma_start(
                        out=wc[:, :, :CK], in_=wsp[fidx][:, :, ck0:ck0 + CK]
                    )
                    gate_inplace(h_t[:, :, ck0:ck0 + CK], wc[:, :, :CK])
                    proj_blocks(h_t[:, :, ck0:ck0 + CK], fidx, ck0, CK)
                    ck0 += CK

    nc.compile()
    return nc


# ---------------------------------------------------------------- host side
def _prep_weights(fc_w, fc_b, fc1_w, fc1_b, fc2_w, fc2_b,
                  rw1_w, rw1_b, rw2_w, rw2_b, proj_w, proj_b):
    f32 = np.float32

    # padded-576 C layout: padded row q = 64*g + s  <->  channel c = 57*g + s
    qof = np.full((CP,), -1, np.int64)
    for g, nch, c0, _ in _c_groups():
        qof[GPAD * g:GPAD * g + nch] = np.arange(c0, c0 + nch)
    qvalid = qof >= 0
    qidx = np.where(qvalid, np.maximum(qof, 0), 0)

    def cols_to_padded576(m):  # [R, C] -> [R, CP] with zero pad cols
        out = np.zeros((m.shape[0], CP), f32)
        out[:, qvalid] = m[:, qidx[qvalid]]
        return out

    def rows_to_padded576(m):  # [C, N] -> [CP, N] with zero pad rows
        out = np.zeros((CP, m.shape[1]), f32)
        out[qvalid] = m[qidx[qvalid]]
        return out

    def vec_to_padded576(v):
        out = np.zeros((CP,), f32)
        out[qvalid] = v[qidx[qvalid]]
        return out

    # fc: columns permuted into 9 HID-groups of 114 (112 for g=8), pad to 128
    fcwp = np.zeros((C, NG * 128), f32)
    fcbp = np.zeros((NG * 128,), f32)
    for g in range(NG):
        n = min(GS_HID * (g + 1), HID) - GS_HID * g
        fcwp[:, 128 * g:128 * g + n] = fc_w[:, GS_HID * g:GS_HID * g + n]
        fcbp[128 * g:128 * g + n] = fc_b[GS_HID * g:GS_HID * g + n]
    fcw_h = np.ascontiguousarray(
        fcwp.reshape(CCB, 128, NG * 128).transpose(1, 0, 2)
    ).astype(BF16_NP)
    fcb_h = np.ascontiguousarray(fcbp.reshape(NG, 128).T).astype(f32)

    def hid_rows_grouped(wm):  # [HID, CP] -> [128, NG, CP] padded group rows
        wp = np.zeros((NG * 128, wm.shape[1]), f32)
        for g in range(NG):
            n = min(GS_HID * (g + 1), HID) - GS_HID * g
            wp[128 * g:128 * g + n] = wm[GS_HID * g:GS_HID * g + n]
        return np.ascontiguousarray(
            wp.reshape(NG, 128, wm.shape[1]).transpose(1, 0, 2)
        ).astype(BF16_NP)

    fc1w_h = hid_rows_grouped(cols_to_padded576(fc1_w))
    fc2w_h = hid_rows_grouped(cols_to_padded576(fc2_w))

    fc1bp = vec_to_padded576(fc1_b)
    fc2bp = vec_to_padded576(fc2_b)
    padb = np.zeros((YCB * 128,), f32)
    fc1b_h = padb.copy(); fc1b_h[:CP] = fc1bp
    fc1b_h = np.ascontiguousarray(fc1b_h.reshape(YCB, 128).T).astype(f32)
    fc2b_h = padb.copy(); fc2b_h[:CP] = fc2bp
    fc2b_h = np.ascontiguousarray(fc2b_h.reshape(YCB, 128).T).astype(f32)

    # proj: rows in padded-576 layout (pad rows zero), cols plain C
    projwp = np.zeros((YCB * 128, C), f32)
    projwp[:CP] = rows_to_padded576(proj_w)
    projw_h = np.ascontiguousarray(
        projwp.reshape(YCB, 128, C).transpose(1, 0, 2)
    ).astype(BF16_NP)
    projb_h = np.ascontiguousarray(
        np.broadcast_to(proj_b[None, :], (128, C))
    ).astype(f32)

    # rw1: rows in padded-576 layout, scaled by 1/MEAN_N (folds the mean)
    rw1p = np.zeros((YCB * 128, C // 4), f32)
    rw1p[:CP] = rows_to_padded576(rw1_w / MEAN_N)
    rw1w_h = np.ascontiguousarray(
        rw1p.reshape(YCB, 128, C // 4).transpose(1, 0, 2)
    ).astype(BF16_NP)
    rw1b_h = np.ascontiguousarray(rw1_b[:, None]).astype(f32)

    # rw2 columns: stream-0 logits in padded cols [0, CP), stream-1 logits in
    # padded cols [YCB*128, YCB*128 + CP) — so the device's 128-wide M-blocks
    # 0..4 are stream 0 and 5..9 are stream 1.
    NQ = YCB * 128
    rw2p = np.zeros((128, 2 * NQ), f32)
    rw2p[:, 0:CP][:, qvalid] = rw2_w[:, 2 * qidx[qvalid]]
    rw2p[:, NQ:NQ + CP][:, qvalid] = rw2_w[:, 2 * qidx[qvalid] + 1]
    rw2w_h = np.ascontiguousarray(rw2p).astype(BF16_NP)
    rw2b_full = np.zeros((2 * NQ,), f32)
    rw2b_full[0:CP][qvalid] = rw2_b[2 * qidx[qvalid]]
    rw2b_full[NQ:NQ + CP][qvalid] = rw2_b[2 * qidx[qvalid] + 1]
    rw2b_h = np.ascontiguousarray(rw2b_full.reshape(2 * YCB, 128).T).astype(f32)

    return dict(
        fcw=fcw_h, fcb=fcb_h, fc1w=fc1w_h, fc1b=fc1b_h, fc2w=fc2w_h,
        fc2b=fc2b_h, projw=projw_h, projb=projb_h, rw1w=rw1w_h, rw1b=rw1b_h,
        rw2w=rw2w_h, rw2b=rw2b_h,
    )


def _get_nc():
    if "nc" not in _CACHE:
        _CACHE["nc"] = build_nc()
    return _CACHE["nc"]


def run(inputs, trace=False, trace_kwargs=None):
    """Run the SPMD kernel; returns (full_output, BassKernelResults)."""
    x = np.asarray(inputs["x"], np.float32)
    shared = _prep_weights(
        np.asarray(inputs["fc_w"], np.float32), np.asarray(inputs["fc_b"], np.float32),
        np.asarray(inputs["fc1_w"], np.float32), np.asarray(inputs["fc1_b"], np.float32),
        np.asarray(inputs["fc2_w"], np.float32), np.asarray(inputs["fc2_b"], np.float32),
        np.asarray(inputs["rw1_w"], np.float32), np.asarray(inputs["rw1_b"], np.float32),
        np.asarray(inputs["rw2_w"], np.float32), np.asarray(inputs["rw2_b"], np.float32),
        np.asarray(inputs["proj_w"], np.float32), np.asarray(inputs["proj_b"], np.float32),
    )

    xf = x.reshape(B * T, HWTOK, C)
    in_maps = []
    for c in range(NCORES):
        sh = xf[NF * c:NF * (c + 1)]                      # [NF, 3136, 512]
        xt = sh.transpose(0, 2, 1).reshape(NF, CCB, 128, HWTOK)
        xt = np.ascontiguousarray(xt.transpose(0, 2, 1, 3)).astype(BF16_NP)
        bm = np.zeros((128, B), np.float32)
        bm[:, (NF * c) // T] = 1.0
        m = dict(shared)
        m["xT"] = xt
        m["bmask"] = bm
        in_maps.append(m)

    nc = _get_nc()
    res = run_bass_kernel_spmd(
        nc, in_maps, list(range(NCORES)),
        trace=trace, **(dict(trace_kwargs=trace_kwargs) if trace_kwargs else {}),
    )

    out = np.empty((B * T, HWTOK, C), np.float32)
    for c in range(NCORES):
        out[NF * c:NF * (c + 1)] = res.results[c]["out"]
    return out.reshape(B, T, H, W, C), res


def kernel(**inputs) -> np.ndarray:
    full, _ = run(inputs, trace=False)
    return full


# revision 11
# speedup vs baseline: 1.2758x; 1.0347x over previous
"""Trainium2 Bass kernel for nn_Mlp_cnn_shift (dense CNN MLP with 3x3 patch-shift
and a softmax-gated mix of two branches).

Strategy
--------
Data-parallel over the 16 (B,T) frames: each of the 8 NeuronCores processes 2
frames end-to-end.  All activations are kept channel-major ([C, tokens]) so the
channel contraction of every matmul has K on partitions, and `x` is
pre-transposed/cast on the host so no on-device transpose is needed.

Patch-shift handling:
 * forward shift (on xh, HID=1024): xh is stored in a zero-padded token layout
   (row pitch 57 = 56 cols + 1 zero pad col, 58-token zero guards per frame)
   and in 9 channel groups of 114 padded to 128 partitions each (host-permuted
   fc_w columns / fc1_w+fc2_w rows).  Every (dh,dw) roll then becomes a pure
   token offset in the fc1 matmul's rhs access pattern, with the zero padding
   reproducing the reference's zero-fill boundary exactly.
 * inverse shift (on gelu(y), C=512): y's channels are produced in 9 groups of
   57 padded to 64 partitions (576 rows = 4.5 blocks; host-permuted fc1_w
   columns), so each group starts at partition 0 or 64 (the HW requires
   compute-engine APs to start at 32-aligned partitions).  The gelu PSUM
   evacuation then writes each group directly into h at its inversely-shifted,
   edge-clipped token positions — the shift costs no extra passes.
   w / the gate / proj all use the same padded-576 channel layout (again via
   host-side weight permutation); padded rows are exactly zero throughout.

The only cross-core coupling is the global (T,H,W) mean feeding the softmax
gate.  It is done as TWO tiny AllReduces (one per frame): the first is
triggered halfway through the kernel and absorbs the cross-core launch skew
under frame-1 compute, so only the second's ~10us floor is exposed.

bf16 matmuls with f32 PSUM accumulation; output f32.  Frame 0's h/w branches
spill to DRAM (bf16) and stream back during the output phase to fit SBUF.
"""

import os
import sys

for _p in ("/opt/trn_rl_repo",):
    if os.path.isdir(_p) and _p not in sys.path:
        sys.path.append(_p)

import numpy as np
import ml_dtypes

import concourse.bass as bass  # noqa: F401
import concourse.mybir as mybir
import concourse.tile as tile
from concourse import bacc
from concourse.bass_utils import run_bass_kernel_spmd

# ---------------------------------------------------------------- constants
SHIFTS = [(1, 1), (1, 0), (1, -1), (0, 1), (0, 0), (0, -1), (-1, 1), (-1, 0), (-1, -1)]
NG = 9
B, T, H, W, C = 2, 8, 56, 56, 512
HID = 1024
NCORES = 8
NF = (B * T) // NCORES          # frames per core = 2
HWTOK = H * W                   # 3136 tokens per frame
RP = W + 1                      # padded row pitch = 57
GUARD = RP + 1                  # 58 zero tokens on each end
FRPAD = RP * H                  # 3192
XHSPAN = GUARD + FRPAD + GUARD  # 3308
RG = 7                          # row groups per frame
RGR = H // RG                   # 8 rows per group
RGT = RGR * W                   # 448 valid tokens per row group
RGP = RGR * RP                  # 456 padded tokens per row group
GS_HID = 114                    # hid shift-group size (9*114 = 1026 >= 1024)
GS_C = 57                       # C shift-group size (9*57 = 513 >= 512)
GPAD = 64                       # C shift groups padded to 64 partitions
CP = NG * GPAD                  # 576 padded C rows
YCB = (CP + 127) // 128         # 5 row-blocks (last half-used)
CCB = C // 128                  # 4
HCB = HID // 128                # 8
MEAN_N = float(T * H * W)

F32 = mybir.dt.float32
BF16 = mybir.dt.bfloat16
BF16_NP = ml_dtypes.bfloat16

_CACHE = {}


def _c_groups():
    """(g, n_ch, real channel range) for the 9 C shift groups."""
    out = []
    for g in range(NG):
        c0 = GS_C * g
        c1 = min(GS_C * (g + 1), C)
        out.append((g, c1 - c0, c0, c1))
    return out


# ---------------------------------------------------------------- device kernel
def build_nc():
    nc = bacc.Bacc("TRN2", target_bir_lowering=False, debug=False, num_devices=NCORES)

    dp = nc.declare_dram_parameter
    xT = dp("xT", [NF, 128, CCB, HWTOK], BF16, isOutput=False)
    fcw = dp("fcw", [128, CCB, NG * 128], BF16, isOutput=False)
    fcb = dp("fcb", [128, NG], F32, isOutput=False)
    fc1w = dp("fc1w", [128, NG, CP], BF16, isOutput=False)
    fc1b = dp("fc1b", [128, YCB], F32, isOutput=False)
    fc2w = dp("fc2w", [128, NG, CP], BF16, isOutput=False)
    fc2b = dp("fc2b", [128, YCB], F32, isOutput=False)
    projw = dp("projw", [128, YCB, C], BF16, isOutput=False)
    projb = dp("projb", [128, C], F32, isOutput=False)
    rw1w = dp("rw1w", [128, YCB, 128], BF16, isOutput=False)
    rw1b = dp("rw1b", [128, 1], F32, isOutput=False)
    rw2w = dp("rw2w", [128, 2 * YCB * 128], BF16, isOutput=False)
    rw2b = dp("rw2b", [128, 2 * YCB], F32, isOutput=False)
    bmask = dp("bmask", [128, B], F32, isOutput=False)
    out_d = dp("out", [NF, HWTOK, C], F32, isOutput=True)

    # spill space for the w branch of each frame + collective bounce buffers
    wsp = [nc.dram_tensor(f"wsp{f}", [128, YCB, HWTOK], BF16) for f in range(NF)]
    ccin = [nc.dram_tensor(f"ccin{f}", [B, 128, YCB], F32) for f in range(NF)]
    ccout = [
        nc.dram_tensor(f"ccout{f}", [B, 128, YCB], F32, addr_space="Shared")
        for f in range(NF)
    ]

    AF = mybir.ActivationFunctionType
    ALU = mybir.AluOpType

    with tile.TileContext(nc, num_cores=NCORES) as tc:
        with (
            tc.tile_pool(name="singles", bufs=1) as singles,
            tc.tile_pool(name="xh_pool", bufs=1) as xh_pool,
            tc.tile_pool(name="h_pool", bufs=2) as h_pool,
            tc.tile_pool(name="w_pool", bufs=2) as w_pool,
            tc.tile_pool(name="xt_pool", bufs=2) as xt_pool,
            tc.tile_pool(name="ostage", bufs=3) as ostage,
            tc.tile_pool(name="dstream", bufs=3) as dstream,
            tc.tile_pool(name="small", bufs=1) as small,
            tc.tile_pool(name="mmpsum", bufs=6, space="PSUM") as mmpsum,
            tc.tile_pool(name="gpsum", bufs=1, space="PSUM") as gpsum,
        ):
            # ---- load weights (resident for the whole kernel)
            def load(name, shape, dtype, src):
                t = singles.tile(shape, dtype, name=name)
                nc.sync.dma_start(out=t, in_=src[:])
                return t

            # only what frame-0's fc pass needs is loaded up front; the rest
            # loads while it runs (keeps the kernel head short)
            fcw_s = load("fcw_s", [128, CCB, NG * 128], BF16, fcw)
            fcb_s = load("fcb_s", [128, NG], F32, fcb)
            _rest = {}

            def load_rest():
                _rest["fc1w_s"] = load("fc1w_s", [128, NG, CP], BF16, fc1w)
                _rest["fc1b_s"] = load("fc1b_s", [128, YCB], F32, fc1b)
                _rest["fc2w_s"] = load("fc2w_s", [128, NG, CP], BF16, fc2w)
                _rest["fc2b_s"] = load("fc2b_s", [128, YCB], F32, fc2b)
                _rest["projw_s"] = load("projw_s", [128, YCB, C], BF16, projw)
                _rest["projb_s"] = load("projb_s", [128, C], F32, projb)
                _rest["rw1w_s"] = load("rw1w_s", [128, YCB, 128], BF16, rw1w)
                _rest["rw1b_s"] = load("rw1b_s", [128, 1], F32, rw1b)
                _rest["rw2w_s"] = load("rw2w_s", [128, 2 * YCB * 128], BF16, rw2w)
                _rest["rw2b_s"] = load("rw2b_s", [128, 2 * YCB], F32, rw2b)
                _rest["bmask_s"] = load("bmask_s", [128, B], F32, bmask)

            a0_s = singles.tile([128, YCB], F32)   # gate for the h branch

            # xh, padded token layout, persistent across frames.
            xh = xh_pool.tile([128, NG, XHSPAN], BF16)
            # zero guards + per-row pad column once; the body is fully
            # rewritten by every frame's fc pass.
            nc.vector.memset(xh[:, :, :GUARD], 0.0)
            nc.vector.memset(xh[:, :, GUARD + FRPAD:], 0.0)
            xh_rows = xh[:, :, GUARD:GUARD + FRPAD].rearrange(
                "p g (r c) -> p g r c", c=RP
            )
            nc.vector.memset(xh_rows[:, :, :, W:], 0.0)

            hw_tiles = []
            part_sums = []

            for f in range(NF):
                # ---------------- A: xh = gelu(x @ fc_w + fc_b), group-blocked
                for rg in range(RG):
                    xt_t = xt_pool.tile([128, CCB, RGT], BF16, tag="xt")
                    nc.sync.dma_start(
                        out=xt_t, in_=xT[f, :, :, rg * RGT:(rg + 1) * RGT]
                    )
                    for mb in range(NG):
                        ps = mmpsum.tile([128, 512], F32, tag="mm")
                        for k in range(CCB):
                            nc.tensor.matmul(
                                ps[:, :RGT],
                                lhsT=fcw_s[:, k, mb * 128:(mb + 1) * 128],
                                rhs=xt_t[:, k, :],
                                start=(k == 0),
                                stop=(k == CCB - 1),
                            )
                        dst = xh[
                            :, mb, GUARD + rg * RGP:GUARD + (rg + 1) * RGP
                        ].rearrange("p (r c) -> p r c", c=RP)[:, :, :W]
                        src = ps[:, :RGT].rearrange("p (r c) -> p r c", c=W)
                        nc.scalar.activation(
                            out=dst, in_=src, func=AF.Gelu,
                            bias=fcb_s[:, mb:mb + 1],
                        )

                if f == 0:
                    # frame-0 fc pass is in flight; now bring in the rest
                    load_rest()
                    fc1w_s = _rest["fc1w_s"]; fc1b_s = _rest["fc1b_s"]
                    fc2w_s = _rest["fc2w_s"]; fc2b_s = _rest["fc2b_s"]
                    projw_s = _rest["projw_s"]; projb_s = _rest["projb_s"]
                    rw1w_s = _rest["rw1w_s"]; rw1b_s = _rest["rw1b_s"]
                    rw2w_s = _rest["rw2w_s"]; rw2b_s = _rest["rw2b_s"]
                    bmask_s = _rest["bmask_s"]

                # ---------------- C: h = invshift(gelu(shift(xh) @ fc1_w + b))
                # y channels live in 9 groups of 57 padded to 64 partitions
                # (576 rows = YCB blocks); the inverse shift is applied by the
                # gelu evacuation writing each group at shifted positions.
                h_t = h_pool.tile([128, YCB, HWTOK], BF16, tag="h")
                nc.gpsimd.memset(h_t[:], 0.0)
                h4 = h_t.rearrange("p c (i j) -> p c i j", j=W)
                hsum_st = small.tile([128, YCB, RG], F32, tag=f"hsst{f}")
                nc.vector.memset(hsum_st[:], 0.0)
                wsum_st = small.tile([128, YCB, RG], F32, tag=f"wsst{f}")
                nc.vector.memset(wsum_st[:], 0.0)
                for rg in range(RG):
                    for mb in range(YCB):
                        M = min(128, CP - mb * 128)
                        ps = mmpsum.tile([128, 512], F32, tag="mm")
                        for g in range(NG):
                            off = -(SHIFTS[g][0] * RP + SHIFTS[g][1])
                            s0 = GUARD + rg * RGP + off
                            rhs2 = xh[:, g, s0:s0 + RGP].rearrange(
                                "p (r c) -> p r c", c=RP
                            )[:, :, :W]
                            nc.tensor.matmul(
                                ps[:M, :RGT],
                                lhsT=fc1w_s[:, g, mb * 128:mb * 128 + M],
                                rhs=rhs2,
                                start=(g == 0),
                                stop=(g == NG - 1),
                            )
                        ps3 = ps[:, :RGT].rearrange("p (r c) -> p r c", c=W)
                        # two 64-partition group-halves per block (block 4:
                        # only the lower half carries group 8)
                        for half in range(2):
                            q0 = mb * 128 + half * GPAD
                            g = q0 // GPAD
                            if g >= NG:
                                continue
                            nch = min(GS_C * (g + 1), C) - GS_C * g
                            sh, sw = SHIFTS[g]
                            # h(i',j') = gelu_y(i'+sh, j'+sw); this window
                            # holds gelu_y rows [8rg, 8rg+8)
                            i0 = max(0, 8 * rg - sh)
                            i1 = min(H, 8 * rg + 8 - sh)
                            j0, j1 = max(0, -sw), min(W, W - sw)
                            p0 = half * GPAD
                            nc.scalar.activation(
                                out=h4[p0:p0 + nch, mb, i0:i1, j0:j1],
                                in_=ps3[
                                    p0:p0 + nch,
                                    i0 + sh - 8 * rg:i1 + sh - 8 * rg,
                                    j0 + sw:j1 + sw,
                                ],
                                func=AF.Gelu,
                                bias=fc1b_s[p0:p0 + nch, mb:mb + 1],
                                accum_out=hsum_st[p0:p0 + nch, mb, rg:rg + 1],
                            )

                # ---------------- B: w = gelu(xh @ fc2_w + fc2_b), padded-576,
                # built per row-group and spilled to DRAM (streamed back in D)
                for rg in range(RG):
                    w_rg = w_pool.tile([128, YCB, RGT], BF16, tag="wrg")
                    if True:
                        nc.vector.memset(w_rg[GPAD:, YCB - 1, :], 0.0)
                    for mb in range(YCB):
                        M = min(128, CP - mb * 128)
                        ps = mmpsum.tile([128, 512], F32, tag="mm")
                        for g in range(NG):
                            s0 = GUARD + rg * RGP
                            rhs2 = xh[:, g, s0:s0 + RGP].rearrange(
                                "p (r c) -> p r c", c=RP
                            )[:, :, :W]
                            nc.tensor.matmul(
                                ps[:M, :RGT],
                                lhsT=fc2w_s[:, g, mb * 128:mb * 128 + M],
                                rhs=rhs2,
                                start=(g == 0),
                                stop=(g == NG - 1),
                            )
                        dst = w_rg[:M, mb, :].rearrange("p (r c) -> p r c", c=W)
                        srcp = ps[:M, :RGT].rearrange("p (r c) -> p r c", c=W)
                        nc.scalar.activation(
                            out=dst, in_=srcp, func=AF.Gelu,
                            bias=fc2b_s[:M, mb:mb + 1],
                            accum_out=wsum_st[:M, mb, rg:rg + 1],
                        )
                    nc.sync.dma_start(
                        out=wsp[f][:, :, rg * RGT:(rg + 1) * RGT], in_=w_rg[:]
                    )

                # ---------------- per-frame gate partial sum + AllReduce
                hs = small.tile([128, YCB], F32, tag=f"hs{f}")
                nc.vector.tensor_reduce(
                    out=hs, in_=hsum_st[:], axis=mybir.AxisListType.X, op=ALU.add
                )
                ws = small.tile([128, YCB], F32, tag=f"ws{f}")
                nc.vector.tensor_reduce(
                    out=ws, in_=wsum_st[:], axis=mybir.AxisListType.X, op=ALU.add
                )
                part = small.tile([128, YCB], F32, tag=f"part{f}")
                nc.vector.tensor_tensor(part, hs, ws, ALU.add)
                part_sums.append(part)
                # mask into the own-batch row and AllReduce; frame 0's
                # collective overlaps frame 1's compute (and absorbs the
                # cross-core launch skew).
                t0 = small.tile([128, YCB], F32, tag=f"cca{f}")
                nc.vector.tensor_scalar_mul(t0, part, bmask_s[:, 0:1])
                t1 = small.tile([128, YCB], F32, tag=f"ccb{f}")
                nc.vector.tensor_scalar_mul(t1, part, bmask_s[:, 1:2])
                nc.sync.dma_start(out=ccin[f][0], in_=t0)
                nc.sync.dma_start(out=ccin[f][1], in_=t1)
                nc.gpsimd.collective_compute(
                    "AllReduce",
                    ALU.add,
                    replica_groups=[list(range(NCORES))],
                    ins=[ccin[f][:]],
                    outs=[ccout[f][:]],
                )

                hw_tiles.append(h_t)

            # keep TensorE's activity monitor warm across the second
            # AllReduce's latency window (junk matmuls, results unread) —
            # otherwise the whole output phase runs at the 4/8 cold clock
            for wi in range(75):
                wp = mmpsum.tile([128, 512], F32, tag="mm", name=f"warm{wi}")
                nc.tensor.matmul(
                    wp[:, :512],
                    lhsT=fcw_s[:, 0, 0:128],
                    rhs=fcw_s[:, 1, 0:512],
                    start=True,
                    stop=True,
                )

            # ---------------- combine the two AllReduce results -> z
            acc = []
            for f in range(NF):
                za = small.tile([128, YCB], F32, tag=f"za{f}")
                nc.sync.dma_start(out=za, in_=ccout[f][0])
                zb = small.tile([128, YCB], F32, tag=f"zb{f}")
                nc.sync.dma_start(out=zb, in_=ccout[f][1])
                nc.vector.tensor_scalar_mul(za, za, bmask_s[:, 0:1])
                nc.vector.tensor_scalar_mul(zb, zb, bmask_s[:, 1:2])
                s = small.tile([128, YCB], F32, tag=f"zs{f}")
                nc.vector.tensor_tensor(s, za, zb, ALU.add)
                acc.append(s)
            zsum = small.tile([128, YCB], F32, tag="zsum")
            nc.vector.tensor_tensor(zsum, acc[0], acc[1], ALU.add)
            zbf = small.tile([128, YCB], BF16, tag="zbf")
            nc.vector.tensor_copy(out=zbf, in_=zsum)

            # ---------------- gate: a = softmax over the 2 streams
            # (1/MEAN_N is folded into rw1w on the host)
            psg = gpsum.tile([128, 1], F32, tag="psg")
            for k in range(YCB):
                nc.tensor.matmul(
                    psg,
                    lhsT=rw1w_s[:, k, :],
                    rhs=zbf[:, k:k + 1],
                    start=(k == 0),
                    stop=(k == YCB - 1),
                )
            gv = small.tile([128, 1], BF16, tag="gv")
            nc.scalar.activation(out=gv, in_=psg, func=AF.Gelu, bias=rw1b_s[:, 0:1])
            psu = gpsum.tile([128, 2 * YCB], F32, tag="psu")
            for m in range(2 * YCB):
                nc.tensor.matmul(
                    psu[:, m:m + 1],
                    lhsT=rw2w_s[:, m * 128:(m + 1) * 128],
                    rhs=gv,
                    start=True,
                    stop=True,
                )
            uv = small.tile([128, 2 * YCB], F32, tag="uv")
            nc.vector.tensor_tensor(uv, psu, rw2b_s, ALU.add)
            l0, l1 = uv[:, 0:YCB], uv[:, YCB:2 * YCB]
            mx = small.tile([128, YCB], F32, tag="mx")
            nc.vector.tensor_tensor(mx, l0, l1, ALU.max)
            d0 = small.tile([128, YCB], F32, tag="d0")
            nc.vector.tensor_tensor(d0, l0, mx, ALU.subtract)
            d1 = small.tile([128, YCB], F32, tag="d1")
            nc.vector.tensor_tensor(d1, l1, mx, ALU.subtract)
            e0 = small.tile([128, YCB], F32, tag="e0")
            nc.scalar.activation(out=e0, in_=d0, func=AF.Exp)
            e1 = small.tile([128, YCB], F32, tag="e1")
            nc.scalar.activation(out=e1, in_=d1, func=AF.Exp)
            esum = small.tile([128, YCB], F32, tag="esum")
            nc.vector.tensor_tensor(esum, e0, e1, ALU.add)
            rec = small.tile([128, YCB], F32, tag="rec")
            nc.vector.reciprocal(rec, esum)
            nc.vector.tensor_tensor(a0_s, e0, rec, ALU.mult)

            # ---------------- D: out = (a0*h + (1-a0)*w) @ proj_w + proj_b
            def proj_blocks(gated_ap, fidx, tbase, ntok):
                """gated_ap: [128, YCB, ntok] bf16 SBUF ap (padded-576)."""
                m0 = 0
                while m0 < ntok:
                    M = min(128, ntok - m0)
                    pp = mmpsum.tile([128, 512], F32, tag="mm")
                    for kb in range(YCB):
                        nc.tensor.matmul(
                            pp[:M, :C],
                            lhsT=gated_ap[:, kb, m0:m0 + M],
                            rhs=projw_s[:, kb, :],
                            start=(kb == 0),
                            stop=(kb == YCB - 1),
                        )
                    ot = ostage.tile([128, C], F32, tag="ot")
                    nc.vector.tensor_tensor(ot[:M], pp[:M, :C], projb_s[:M], ALU.add)
                    nc.sync.dma_start(
                        out=out_d[fidx, tbase + m0:tbase + m0 + M, :], in_=ot[:M]
                    )
                    m0 += M

            def gate_inplace(h_ap, w_ap):
                """h_ap <- a0*h + (1-a0)*w   (= w + a0*(h-w)), in place.
                Per row-block chains so proj passes can start early."""
                for kb in range(YCB):
                    nc.vector.tensor_tensor(
                        h_ap[:, kb], h_ap[:, kb], w_ap[:, kb], ALU.subtract
                    )
                    nc.scalar.activation(
                        out=h_ap[:, kb], in_=h_ap[:, kb],
                        func=AF.Copy, scale=a0_s[:, kb:kb + 1],
                    )
                    nc.vector.tensor_tensor(
                        h_ap[:, kb], h_ap[:, kb], w_ap[:, kb], ALU.add
                    )

            # h is resident for both frames; stream each frame's w back in
            # 512-token chunks, gate in place on the h slice, then project.
            for fidx in (1, 0):
                h_t = hw_tiles[fidx]
                ck0 = 0
                while ck0 < HWTOK:
                    CK = min(512, HWTOK - ck0)
                    wc = dstream.tile([128, YCB, 512], BF16, tag="wc")
                    nc.sync.dma_start(
                        out=wc[:, :, :CK], in_=wsp[fidx][:, :, ck0:ck0 + CK]
                    )
                    gate_inplace(h_t[:, :, ck0:ck0 + CK], wc[:, :, :CK])
                    proj_blocks(h_t[:, :, ck0:ck0 + CK], fidx, ck0, CK)
                    ck0 += CK

    nc.compile()
    return nc


# ---------------------------------------------------------------- host side
def _prep_weights(fc_w, fc_b, fc1_w, fc1_b, fc2_w, fc2_b,
                  rw1_w, rw1_b, rw2_w, rw2_b, proj_w, proj_b):
    f32 = np.float32

    # padded-576 C layout: padded row q = 64*g + s  <->  channel c = 57*g + s
    qof = np.full((CP,), -1, np.int64)
    for g, nch, c0, _ in _c_groups():
        qof[GPAD * g:GPAD * g + nch] = np.arange(c0, c0 + nch)
    qvalid = qof >= 0
    qidx = np.where(qvalid, np.maximum(qof, 0), 0)

    def cols_to_padded576(m):  # [R, C] -> [R, CP] with zero pad cols
        out = np.zeros((m.shape[0], CP), f32)
        out[:, qvalid] = m[:, qidx[qvalid]]
        return out

    def rows_to_padded576(m):  # [C, N] -> [CP, N] with zero pad rows
        out = np.zeros((CP, m.shape[1]), f32)
        out[qvalid] = m[qidx[qvalid]]
        return out

    def vec_to_padded576(v):
        out = np.zeros((CP,), f32)
        out[qvalid] = v[qidx[qvalid]]
        return out

    # fc: columns permuted into 9 HID-groups of 114 (112 for g=8), pad to 128
    fcwp = np.zeros((C, NG * 128), f32)
    fcbp = np.zeros((NG * 128,), f32)
    for g in range(NG):
        n = min(GS_HID * (g + 1), HID) - GS_HID * g
        fcwp[:, 128 * g:128 * g + n] = fc_w[:, GS_HID * g:GS_HID * g + n]
        fcbp[128 * g:128 * g + n] = fc_b[GS_HID * g:GS_HID * g + n]
    fcw_h = np.ascontiguousarray(
        fcwp.reshape(CCB, 128, NG * 128).transpose(1, 0, 2)
    ).astype(BF16_NP)
    fcb_h = np.ascontiguousarray(fcbp.reshape(NG, 128).T).astype(f32)

    def hid_rows_grouped(wm):  # [HID, CP] -> [128, NG, CP] padded group rows
        wp = np.zeros((NG * 128, wm.shape[1]), f32)
        for g in range(NG):
            n = min(GS_HID * (g + 1), HID) - GS_HID * g
            wp[128 * g:128 * g + n] = wm[GS_HID * g:GS_HID * g + n]
        return np.ascontiguousarray(
            wp.reshape(NG, 128, wm.shape[1]).transpose(1, 0, 2)
        ).astype(BF16_NP)

    fc1w_h = hid_rows_grouped(cols_to_padded576(fc1_w))
    fc2w_h = hid_rows_grouped(cols_to_padded576(fc2_w))

    fc1bp = vec_to_padded576(fc1_b)
    fc2bp = vec_to_padded576(fc2_b)
    padb = np.zeros((YCB * 128,), f32)
    fc1b_h = padb.copy(); fc1b_h[:CP] = fc1bp
    fc1b_h = np.ascontiguousarray(fc1b_h.reshape(YCB, 128).T).astype(f32)
    fc2b_h = padb.copy(); fc2b_h[:CP] = fc2bp
    fc2b_h = np.ascontiguousarray(fc2b_h.reshape(YCB, 128).T).astype(f32)

    # proj: rows in padded-576 layout (pad rows zero), cols plain C
    projwp = np.zeros((YCB * 128, C), f32)
    projwp[:CP] = rows_to_padded576(proj_w)
    projw_h = np.ascontiguousarray(
        projwp.reshape(YCB, 128, C).transpose(1, 0, 2)
    ).astype(BF16_NP)
    projb_h = np.ascontiguousarray(
        np.broadcast_to(proj_b[None, :], (128, C))
    ).astype(f32)

    # rw1: rows in padded-576 layout, scaled by 1/MEAN_N (folds the mean)
    rw1p = np.zeros((YCB * 128, C // 4), f32)
    rw1p[:CP] = rows_to_padded576(rw1_w / MEAN_N)
    rw1w_h = np.ascontiguousarray(
        rw1p.reshape(YCB, 128, C // 4).transpose(1, 0, 2)
    ).astype(BF16_NP)
    rw1b_h = np.ascontiguousarray(rw1_b[:, None]).astype(f32)

    # rw2 columns: stream-0 logits in padded cols [0, CP), stream-1 logits in
    # padded cols [YCB*128, YCB*128 + CP) — so the device's 128-wide M-blocks
    # 0..4 are stream 0 and 5..9 are stream 1.
    NQ = YCB * 128
    rw2p = np.zeros((128, 2 * NQ), f32)
    rw2p[:, 0:CP][:, qvalid] = rw2_w[:, 2 * qidx[qvalid]]
    rw2p[:, NQ:NQ + CP][:, qvalid] = rw2_w[:, 2 * qidx[qvalid] + 1]
    rw2w_h = np.ascontiguousarray(rw2p).astype(BF16_NP)
    rw2b_full = np.zeros((2 * NQ,), f32)
    rw2b_full[0:CP][qvalid] = rw2_b[2 * qidx[qvalid]]
    rw2b_full[NQ:NQ + CP][qvalid] = rw2_b[2 * qidx[qvalid] + 1]
    rw2b_h = np.ascontiguousarray(rw2b_full.reshape(2 * YCB, 128).T).astype(f32)

    return dict(
        fcw=fcw_h, fcb=fcb_h, fc1w=fc1w_h, fc1b=fc1b_h, fc2w=fc2w_h,
        fc2b=fc2b_h, projw=projw_h, projb=projb_h, rw1w=rw1w_h, rw1b=rw1b_h,
        rw2w=rw2w_h, rw2b=rw2b_h,
    )


def _get_nc():
    if "nc" not in _CACHE:
        _CACHE["nc"] = build_nc()
    return _CACHE["nc"]


def run(inputs, trace=False, trace_kwargs=None):
    """Run the SPMD kernel; returns (full_output, BassKernelResults)."""
    x = np.asarray(inputs["x"], np.float32)
    shared = _prep_weights(
        np.asarray(inputs["fc_w"], np.float32), np.asarray(inputs["fc_b"], np.float32),
        np.asarray(inputs["fc1_w"], np.float32), np.asarray(inputs["fc1_b"], np.float32),
        np.asarray(inputs["fc2_w"], np.float32), np.asarray(inputs["fc2_b"], np.float32),
        np.asarray(inputs["rw1_w"], np.float32), np.asarray(inputs["rw1_b"], np.float32),
        np.asarray(inputs["rw2_w"], np.float32), np.asarray(inputs["rw2_b"], np.float32),
        np.asarray(inputs["proj_w"], np.float32), np.asarray(inputs["proj_b"], np.float32),
    )

    xf = x.reshape(B * T, HWTOK, C)
    in_maps = []
    for c in range(NCORES):
        sh = xf[NF * c:NF * (c + 1)]                      # [NF, 3136, 512]
        xt = sh.transpose(0, 2, 1).reshape(NF, CCB, 128, HWTOK)
        xt = np.ascontiguousarray(xt.transpose(0, 2, 1, 3)).astype(BF16_NP)
        bm = np.zeros((128, B), np.float32)
        bm[:, (NF * c) // T] = 1.0
        m = dict(shared)
        m["xT"] = xt
        m["bmask"] = bm
        in_maps.append(m)

    nc = _get_nc()
    res = run_bass_kernel_spmd(
        nc, in_maps, list(range(NCORES)),
        trace=trace, **(dict(trace_kwargs=trace_kwargs) if trace_kwargs else {}),
    )

    out = np.empty((B * T, HWTOK, C), np.float32)
    for c in range(NCORES):
        out[NF * c:NF * (c + 1)] = res.results[c]["out"]
    return out.reshape(B, T, H, W, C), res


def kernel(**inputs) -> np.ndarray:
    full, _ = run(inputs, trace=False)
    return full
